# revision 1
# baseline (speedup 1.0000x reference)
"""Trainium2 Bass kernel for nn_Decoder_64201171141372.

6-layer pre-norm transformer decoder (D=1024, H=16, F=4096, B=8, S=512).
Sharding: data-parallel over batch — each of the 8 NeuronCores computes one
batch element end-to-end; no collectives.

Device-side layout: activations are kept TRANSPOSED in SBUF as [d_part=128,
d_outer, t] so every linear layer runs as matmul(lhsT=W_tile, rhs=xT) with
the contraction dim on partitions. LayerNorm / softmax statistics (which are
per-token, i.e. per free-element) are computed with ones-matmuls and
broadcast back across partitions with rank-1 matmuls. All matmul operands
are bf16 (fp32 PSUM accumulation); the residual stream stays fp32.
"""

import math

import numpy as np
import ml_dtypes

import concourse.bass as bass
import concourse.mybir as mybir
import concourse.tile as tile
from concourse.masks import make_identity
from concourse.vector_clock import ScopedClock, VectorClock

bf16 = ml_dtypes.bfloat16
F32 = mybir.dt.float32
B16 = mybir.dt.bfloat16
I32 = mybir.dt.int32

P = 128
V, D, H, F, L = 32000, 1024, 16, 4096, 6
T = 512            # decoder seq len == encoder seq len
HD = D // H        # 64
DO = D // P        # 8
FO = F // P        # 32
TO = T // P        # 4
NSLOT = 3 * L + 1  # layernorm slots (3 per layer + final)
EPS = 1e-5
N_CORES = 8
N_PROCS = 27

AF = mybir.ActivationFunctionType
OP = mybir.AluOpType

# ---- tuning flags (overridable before build_decoder) ----
LN_PREP_ACT = False    # xbd/x2d copies on ACT (True) or DVE (False)
UB_ACT = True          # attention ub copy on ACT (True) or DVE (False)
CAUSAL_TRIM = True     # restrict causal blocks to un-masked q range
ATTN_PAIR = False      # pair-adjacent score matmuls
FILLERS = True         # interleave cross-K/V into self-attention
LN2_FILL = 2           # cross-K/V pieces held back to fill the LN2 stats bubble
ET_BUFS = 3            # expT pipeline depth
RB_BUFS = 2            # rdb/ub pipeline depth


class _TC(tile.TileContext):
    """TileContext whose exit drain splits sem waits one per instruction.

    The walrus build in this container only encodes a single sync-wait on
    CTRL-class (Drain/NoOp) instructions; the stock tile exit aggregates one
    wait per logical proc onto one Drain and dies in codegen with "Too many
    sync wait commands". Emit one single-wait NOP per proc on the SP queue
    instead; SP program order then makes the final drain safe with no waits.
    """

    def _drain_and_barrier(self, tick_clock, wait_clock):
        gc = tick_clock.global_clock
        for p in range(N_PROCS):
            t = gc[p]
            if t:
                nop = self.nc.sync.nop(nofuse=True)
                pc = VectorClock([t if q == p else 0 for q in range(N_PROCS)])
                wait_clock.add_sem_waits(nop.ins, ScopedClock({None: pc}))
        self.nc.sync.drain()
        self.nc.all_engine_barrier()
        assert self.sems is not None
        popped = self.nc._tile_sem_poison_stack.pop()
        assert popped is self._sem_poison
        self.nc.clear_and_free_semaphores(list(self.sems.allocated().values()))
        self.nc.all_engine_barrier()


def _split_sync_waits(nc, max_waits=1):
    """Walrus in this container encodes at most one sync-wait per instruction.

    Tile's wait assigner attaches one wait per depended-on logical proc; move
    the extras onto single-wait NoOps inserted just before the instruction on
    the same engine queue (program order preserves the sync semantics).
    """
    n_added = 0
    for f in nc.m.functions:
        for bb in f.blocks:
            insts = bb.instructions
            new_list = []
            for ins in insts:
                si = getattr(ins, "sync_info", None)
                waits = list(si.on_wait) if si is not None and si.on_wait else []
                if len(waits) > max_waits:
                    for w in waits[:-max_waits]:
                        nop = mybir.InstNoOp(
                            name=f"I-wsplit{n_added}", ins=[], outs=[])
                        nop.engine = ins.engine
                        nop.sync_info = mybir.SyncInfo(on_wait=[w], on_update=[])
                        new_list.append(nop)
                        n_added += 1
                    ins.sync_info = mybir.SyncInfo(
                        on_wait=waits[-max_waits:], on_update=list(si.on_update))
                new_list.append(ins)
            if n_added:
                insts[:] = new_list
    return n_added


# ---------------------------------------------------------------- packing ---

def _wpack_offsets():
    """Column offsets (in bf16 elements per partition row) into wpack.

    Each weight W[din, dout] is stored as [128, din//128, dout] flattened on
    the free axis; value at [p, ko, n] = W[ko*128 + p, n].
    """
    offs = {}
    c = 0

    def add(name, ko, n):
        nonlocal c
        offs[name] = (c, ko, n)
        c += ko * n

    for l in range(L):
        for nm, ko, n in [
            ("saq", 8, 1024), ("sak", 8, 1024), ("sav", 8, 1024), ("sao", 8, 1024),
            ("caq", 8, 1024), ("cak", 8, 1024), ("cav", 8, 1024), ("cao", 8, 1024),
            ("w1", 8, 4096), ("w2", 32, 1024),
        ]:
            add(f"{nm}{l}", ko, n)
    return offs, c


def _bpack_offsets():
    """Column offsets into bpack [128, cols] f32: bias b[d] at [d%128, off + d//128]."""
    offs = {}
    c = 0

    def add(name, w):
        nonlocal c
        offs[name] = c
        c += w

    for l in range(L):
        for nm, w in [
            ("sabq", 8), ("sabk", 8), ("sabv", 8), ("sabo", 8),
            ("cabq", 8), ("cabk", 8), ("cabv", 8), ("cabo", 8),
            ("b1", 32), ("b2", 8),
        ]:
            add(f"{nm}{l}", w)
    return offs, c


def _col_major(w):
    """[din, n] -> [128, din//128, n] with [p, ko, n] = w[ko*128+p, n]."""
    din, n = w.shape
    return np.ascontiguousarray(w.reshape(din // P, P, n).transpose(1, 0, 2))


def _part_cols(b):
    """[d] -> [128, d//128] with [p, o] = b[o*128+p]."""
    return np.ascontiguousarray(b.reshape(-1, P).T)


def prep_inputs(inputs):
    """Host-side packing: returns (in_maps list for 8 cores)."""
    enc = np.asarray(inputs["encoder_output"], np.float32)       # [8, 512, 1024]
    dec = np.asarray(inputs["decoder_input"]).astype(np.int32)   # [8, 512]
    table = np.ascontiguousarray(np.asarray(inputs["embed_table"], np.float32))
    sa_w = np.asarray(inputs["sa_w"], np.float32)
    sa_b = np.asarray(inputs["sa_b"], np.float32)
    ca_w = np.asarray(inputs["ca_w"], np.float32)
    ca_b = np.asarray(inputs["ca_b"], np.float32)
    w1 = np.asarray(inputs["ffn_w1"], np.float32)
    b1 = np.asarray(inputs["ffn_b1"], np.float32)
    w2 = np.asarray(inputs["ffn_w2"], np.float32)
    b2 = np.asarray(inputs["ffn_b2"], np.float32)
    ln_g = np.asarray(inputs["ln_g"], np.float32)
    ln_b = np.asarray(inputs["ln_b"], np.float32)
    fin_g = np.asarray(inputs["final_g"], np.float32)
    fin_b = np.asarray(inputs["final_b"], np.float32)

    woffs, wcols = _wpack_offsets()
    wpack = np.empty((P, wcols), dtype=bf16)
    for l in range(L):
        for j, nm in enumerate(["saq", "sak", "sav", "sao"]):
            off, ko, n = woffs[f"{nm}{l}"]
            wpack[:, off:off + ko * n] = _col_major(sa_w[l, j]).reshape(P, -1).astype(bf16)
        for j, nm in enumerate(["caq", "cak", "cav", "cao"]):
            off, ko, n = woffs[f"{nm}{l}"]
            wpack[:, off:off + ko * n] = _col_major(ca_w[l, j]).reshape(P, -1).astype(bf16)
        off, ko, n = woffs[f"w1{l}"]
        wpack[:, off:off + ko * n] = _col_major(w1[l]).reshape(P, -1).astype(bf16)
        off, ko, n = woffs[f"w2{l}"]
        wpack[:, off:off + ko * n] = _col_major(w2[l]).reshape(P, -1).astype(bf16)

    boffs, bcols = _bpack_offsets()
    bpack = np.zeros((P, bcols), dtype=np.float32)
    for l in range(L):
        for j, nm in enumerate(["sabq", "sabk", "sabv", "sabo"]):
            bpack[:, boffs[f"{nm}{l}"]:boffs[f"{nm}{l}"] + 8] = _part_cols(sa_b[l, j])
        for j, nm in enumerate(["cabq", "cabk", "cabv", "cabo"]):
            bpack[:, boffs[f"{nm}{l}"]:boffs[f"{nm}{l}"] + 8] = _part_cols(ca_b[l, j])
        bpack[:, boffs[f"b1{l}"]:boffs[f"b1{l}"] + 32] = _part_cols(b1[l])
        bpack[:, boffs[f"b2{l}"]:boffs[f"b2{l}"] + 8] = _part_cols(b2[l])

    gln = np.empty((2, NSLOT, D), dtype=bf16)
    for l in range(L):
        for s in range(3):
            gln[0, 3 * l + s] = ln_g[l, s].astype(bf16)
            gln[1, 3 * l + s] = ln_b[l, s].astype(bf16)
    gln[0, NSLOT - 1] = fin_g.astype(bf16)
    gln[1, NSLOT - 1] = fin_b.astype(bf16)

    # positional encoding, transposed layout [128, 8, 512] fp32
    pos = np.arange(T, dtype=np.float32)[:, None]
    div = np.exp(np.arange(0, D, 2, dtype=np.float32) * (-math.log(10000.0) / D))
    pe = np.zeros((T, D), dtype=np.float32)
    pe[:, 0::2] = np.sin(pos * div)
    pe[:, 1::2] = np.cos(pos * div)
    peT = np.ascontiguousarray(pe.T.reshape(DO, P, T).transpose(1, 0, 2))

    # causal keep-mask in scoresT layout: [p, kto, q] = 1.0 if kto*128+p <= q
    kt_idx = (np.arange(TO * P).reshape(TO, P).T)[:, :, None]   # [128, 4, 1]
    q_idx = np.arange(T)[None, None, :]
    cmask = (kt_idx <= q_idx).astype(bf16)

    in_maps = []
    for c in range(N_CORES):
        encT = np.ascontiguousarray(
            enc[c].T.reshape(DO, P, T).transpose(1, 0, 2)).astype(bf16)
        in_maps.append({
            "wpack": wpack,
            "bpack": bpack,
            "gln": gln,
            "table": table,
            "idx": dec[c].copy(),
            "encT": encT,
            "peT": peT,
            "cmask": cmask,
        })
    return in_maps


def unshard(results):
    """Per-core outT [128, 8, 512] -> full [8, 512, 1024] fp32."""
    out = np.empty((N_CORES, T, D), dtype=np.float32)
    for c in range(N_CORES):
        arr = results[c]["out"]                       # [dp, do, t]
        out[c] = arr.transpose(2, 1, 0).reshape(T, D)  # [t, do*128+dp]
    return out


# ----------------------------------------------------------------- device ---

def build_decoder(repeat: int = 1):
    nc = bass.Bass(trn_type="TRN2")
    woffs, wcols = _wpack_offsets()
    boffs, bcols = _bpack_offsets()

    w_dram = nc.dram_tensor("wpack", [P, wcols], B16, kind="ExternalInput")
    b_dram = nc.dram_tensor("bpack", [P, bcols], F32, kind="ExternalInput")
    gln_dram = nc.dram_tensor("gln", [2, NSLOT, D], B16, kind="ExternalInput")
    table = nc.dram_tensor("table", [V, D], F32, kind="ExternalInput")
    idx_dram = nc.dram_tensor("idx", [T], I32, kind="ExternalInput")
    enc_dram = nc.dram_tensor("encT", [P, DO, T], B16, kind="ExternalInput")
    pe_dram = nc.dram_tensor("peT", [P, DO, T], F32, kind="ExternalInput")
    cm_dram = nc.dram_tensor("cmask", [P, TO, T], B16, kind="ExternalInput")
    out_dram = nc.dram_tensor("out", [P, DO, T], F32, kind="ExternalOutput")

    with _TC(nc) as tc:
        with tc.tile_pool(name="pers", bufs=1) as pers, \
             tc.tile_pool(name="wp", bufs=2) as wp, \
             tc.tile_pool(name="act", bufs=1) as act, \
             tc.tile_pool(name="sc", bufs=2) as scp, \
             tc.tile_pool(name="sm", bufs=1) as sm, \
             tc.tile_pool(name="ps", bufs=8, space="PSUM") as psp:

            # ---- persistent state ----
            x = pers.tile([P, DO, T], F32)       # residual stream (transposed)
            ones = pers.tile([P, T], B16)
            ident = pers.tile([P, P], F32)
            bias_sb = pers.tile([P, bcols], F32)
            enc_sb = pers.tile([P, DO, T], B16)
            cm_sb = pers.tile([P, TO, T], B16)
            idx_sb = pers.tile([P, TO], I32)

            zcol = pers.tile([P, 1], F32)    # zero bias column for ACT ops
            epsc = pers.tile([1, 1], F32)    # eps bias for the LN sqrt
            sA = pers.tile([1, T], B16)      # LN scale row (bf16 rhs for A-mm)
            sB = pers.tile([2, T], B16)      # LN shift row + ones row (B-mm rhs)

            nc.vector.memset(ones[:], 1.0)
            nc.vector.memset(sB[:], 1.0)
            nc.vector.memset(zcol[:], 0.0)
            nc.vector.memset(epsc[:], EPS)
            make_identity(nc, ident[:])
            nc.sync.dma_start(bias_sb[:], b_dram[:])
            nc.sync.dma_start(enc_sb[:], enc_dram[:])
            nc.sync.dma_start(cm_sb[:], cm_dram[:])
            nc.sync.dma_start(idx_sb[:], idx_dram.rearrange("(ti p) -> p ti", p=P))

            def psum(tag="ps"):
                return psp.tile([P, T], F32, tag=tag, name="pt")

            def load_w(name):
                off, ko, n = woffs[name]
                wt = wp.tile([P, 8, 1024], B16, tag="w")
                if n == 4096:  # w1: select a 1024-wide column group q later
                    raise AssertionError("use load_w1")
                src = w_dram[:, off:off + ko * n].rearrange("p (o n) -> p o n", o=ko)
                nc.sync.dma_start(wt[:], src)
                return wt

            def load_w1(l, q):
                off, ko, n = woffs[f"w1{l}"]
                wt = wp.tile([P, 8, 1024], B16, tag="w")
                src = w_dram[:, off:off + ko * n].rearrange("p (o n) -> p o n", o=ko)
                nc.sync.dma_start(wt[:], src[:, :, q * 1024:(q + 1) * 1024])
                return wt

            def load_w2(l, q):
                off, ko, n = woffs[f"w2{l}"]
                wt = wp.tile([P, 8, 1024], B16, tag="w")
                src = w_dram[:, off + q * 8192: off + (q + 1) * 8192]
                nc.sync.dma_start(wt[:], src.rearrange("p (o n) -> p o n", o=8))
                return wt

            # ---------------- layer building blocks ----------------
            def layer_norm(slot, out_t, final=False, fillers=()):
                """out_t[:, do, :] = LN(x) using gln[:, slot]; out dtype = out_t's."""
                s1 = psum()
                s2 = psum()
                # group same-function ACT ops to avoid activation-table thrash
                for g in range(2):
                    xbds, x2ds = [], []
                    for dl in range(4):
                        xbd = scp.tile([P, T], B16, tag="xbd", bufs=4, name="xbd")
                        if LN_PREP_ACT:
                            nc.scalar.copy(xbd[:], x[:, g * 4 + dl, :])
                        else:
                            nc.vector.tensor_copy(xbd[:], x[:, g * 4 + dl, :])
                        xbds.append(xbd)
                    for dl in range(4):
                        x2d = scp.tile([P, T], B16, tag="x2d", bufs=4, name="x2d")
                        if LN_PREP_ACT:
                            nc.scalar.square(x2d[:], x[:, g * 4 + dl, :])
                        else:
                            nc.vector.tensor_tensor(
                                x2d[:], x[:, g * 4 + dl, :], x[:, g * 4 + dl, :],
                                op=OP.mult)
                        x2ds.append(x2d)
                    for dl in range(4):
                        do = g * 4 + dl
                        nc.tensor.matmul(s1[0:1, :], lhsT=ones[:, 0:1],
                                         rhs=xbds[dl][:],
                                         start=(do == 0), stop=(do == DO - 1))
                        nc.tensor.matmul(s2[0:1, :], lhsT=ones[:, 0:1],
                                         rhs=x2ds[dl][:],
                                         start=(do == 0), stop=(do == DO - 1))
                for f in fillers:
                    f()
                m = sm.tile([1, T], F32, tag="m")
                msq = sm.tile([1, T], F32, tag="msq")
                w1t = sm.tile([1, T], F32, tag="w1t")
                S = sm.tile([1, T], F32, tag="S")
                # mean; m^2; var = s2/D - m^2; rstd = 1/sqrt(var+eps)
                nc.vector.tensor_scalar_mul(m[:], s1[0:1, :], 1.0 / D)
                nc.vector.tensor_tensor(msq[:], m[:], m[:], op=OP.mult)
                nc.vector.scalar_tensor_tensor(w1t[:], s2[0:1, :], 1.0 / D, msq[:],
                                               op0=OP.mult, op1=OP.subtract)
                nc.scalar.activation(msq[:], w1t[:], AF.Sqrt, bias=epsc[:])
                nc.vector.reciprocal(S[:], msq[:])
                nc.vector.tensor_copy(sA[:], S[:])
                nc.vector.scalar_tensor_tensor(sB[0:1, :], m[:], -1.0, S[:],
                                               op0=OP.mult, op1=OP.mult)
                gl = sm.tile([2, 1, D], B16, tag="gl")
                nc.sync.dma_start(gl[:], gln_dram[:, slot, :][:, None, :])
                for do in range(DO):
                    A = psum()
                    Bp = psum()
                    nc.tensor.matmul(A[:], lhsT=gl[0:1, 0, do * P:(do + 1) * P],
                                     rhs=sA[:], start=True, stop=True)
                    nc.tensor.matmul(Bp[:], lhsT=gl[0:2, 0, do * P:(do + 1) * P],
                                     rhs=sB[:], start=True, stop=True)
                    tmp = scp.tile([P, T], F32, tag="tmp")
                    nc.vector.tensor_tensor(tmp[:], x[:, do, :], A[:], op=OP.mult)
                    nc.vector.tensor_tensor(out_t[:, do, :], tmp[:], Bp[:], op=OP.add)

            def proj_T(wname, bname, rhs_t, out_t, ko_outer=False):
                """out_t[dout, t] (transposed layout, bf16) = W.T @ rhs + b.

                ko_outer: iterate the contraction dim outermost (groups of 4
                output tiles) so the first matmuls only need rhs slice ko=0 —
                used for the first consumer after a layernorm, whose apply
                produces rhs slices incrementally."""
                wt = load_w(wname)
                boff = boffs[bname]
                if not ko_outer:
                    for do in range(DO):
                        pq = psum()
                        for ko in range(DO):
                            nc.tensor.matmul(pq[:], lhsT=wt[:, ko, do * P:(do + 1) * P],
                                             rhs=rhs_t[:, ko, :],
                                             start=(ko == 0), stop=(ko == DO - 1))
                        nc.scalar.activation(out_t[:, do, :], pq[:], AF.Identity,
                                             bias=bias_sb[:, boff + do:boff + do + 1])
                else:
                    for grp in range(2):
                        pqs = [psum() for _ in range(4)]
                        for ko in range(DO):
                            for dl in range(4):
                                do = grp * 4 + dl
                                nc.tensor.matmul(
                                    pqs[dl][:], lhsT=wt[:, ko, do * P:(do + 1) * P],
                                    rhs=rhs_t[:, ko, :],
                                    start=(ko == 0), stop=(ko == DO - 1))
                        for dl in range(4):
                            do = grp * 4 + dl
                            nc.scalar.activation(
                                out_t[:, do, :], pqs[dl][:], AF.Identity,
                                bias=bias_sb[:, boff + do:boff + do + 1])

            def proj_V(wname, rhs_t, v65_t):
                """v65_t[:, to, h, 0:64] = (rhs.T @ Wv) in natural [t, dout] layout."""
                wt = load_w(wname)
                for to in range(TO):
                    for nh in range(2):
                        pv = psum()
                        for ko in range(DO):
                            nc.tensor.matmul(
                                pv[:], lhsT=rhs_t[:, ko, to * P:(to + 1) * P],
                                rhs=wt[:, ko, nh * 512:(nh + 1) * 512],
                                start=(ko == 0), stop=(ko == DO - 1))
                        nc.vector.tensor_copy(
                            v65_t[:, to, nh * 8:(nh + 1) * 8, 0:64],
                            pv.rearrange("p (h d) -> p h d", d=HD))

            def attention(qt_t, kt_t, v65_t, out_att, causal, bvname,
                          fillers=()):
                """Pipelined per-head (or per-pair) softmax attention."""
                bvoff = boffs[bvname]
                fillers = list(fillers)
                trim = causal and CAUSAL_TRIM

                def q0_of(kto):
                    return kto * P if trim else 0

                def scores_exp(h):
                    """scores + exp (+mask) for one head; returns et."""
                    base = (h % 2) * HD
                    doh = h // 2
                    scs = []
                    for kto in range(TO):
                        q0 = q0_of(kto)
                        sc = psum()
                        nc.tensor.matmul(
                            sc[:, q0:],
                            lhsT=kt_t[base:base + HD, doh, kto * P:(kto + 1) * P],
                            rhs=qt_t[base:base + HD, doh, q0:],
                            start=True, stop=True)
                        scs.append(sc)
                    et = scp.tile([P, TO, T], B16, tag="expT", bufs=ET_BUFS, name="et")
                    for kto in range(TO):
                        q0 = q0_of(kto)
                        nc.scalar.activation(et[:, kto, q0:], scs[kto][:, q0:],
                                             AF.Exp, bias=zcol[:],
                                             scale=1.0 / math.sqrt(HD))
                        if causal:
                            qe = q0 + P if trim else T
                            nc.vector.tensor_tensor(
                                et[:, kto, q0:qe], et[:, kto, q0:qe],
                                cm_sb[:, kto, q0:qe], op=OP.mult)
                    return et

                def pair_scores_exp(pr):
                    """scores + exp for a head pair, score mms pair-adjacent."""
                    et = scp.tile([P, TO, 2, T], B16, tag="expT", bufs=2,
                                  name="et")
                    for kto in range(TO):
                        q0 = q0_of(kto)
                        scs = []
                        for e in range(2):
                            sc = psum()
                            nc.tensor.matmul(
                                sc[:, q0:],
                                lhsT=kt_t[e * HD:(e + 1) * HD, pr,
                                          kto * P:(kto + 1) * P],
                                rhs=qt_t[e * HD:(e + 1) * HD, pr, q0:],
                                start=True, stop=True)
                            scs.append(sc)
                        for e in range(2):
                            nc.scalar.activation(et[:, kto, e, q0:],
                                                 scs[e][:, q0:], AF.Exp,
                                                 bias=zcol[:],
                                                 scale=1.0 / math.sqrt(HD))
                        if causal:
                            qe = q0 + P if trim else T
                            for e in range(2):
                                nc.vector.tensor_tensor(
                                    et[:, kto, e, q0:qe], et[:, kto, e, q0:qe],
                                    cm_sb[:, kto, q0:qe], op=OP.mult)
                    return et

                def emit_ud(h, et_sl):
                    ud = psum()
                    for kto in range(TO):
                        q0 = q0_of(kto)
                        nc.tensor.matmul(ud[0:HD + 1, q0:],
                                         lhsT=v65_t[:, kto, h, :],
                                         rhs=et_sl(kto)[:, q0:],
                                         start=(kto == 0), stop=(kto == TO - 1))
                    return ud

                def emit_recip_ub(ud):
                    rdb = scp.tile([P, T], B16, tag="rdb", bufs=RB_BUFS, name="rdb")
                    with nc.allow_low_precision("softmax denom recip bf16"):
                        nc.vector.reciprocal(rdb[HD:HD + 1, :],
                                             ud[HD:HD + 1, :])
                    ub = scp.tile([P, T], B16, tag="ub", bufs=RB_BUFS, name="ub")
                    if UB_ACT:
                        nc.scalar.activation(ub[0:HD, :], ud[0:HD, :], AF.Copy)
                    else:
                        nc.vector.tensor_copy(ub[0:HD, :], ud[0:HD, :])
                    return rdb, ub

                def emit_norm(h, ub, rdb):
                    base = (h % 2) * HD
                    doh = h // 2
                    rb = psum()
                    nc.tensor.matmul(rb[0:HD, :], lhsT=ones[HD:HD + 1, 0:HD],
                                     rhs=rdb[HD:HD + 1, :], start=True, stop=True)
                    sl = out_att[base:base + HD, doh, :]
                    nc.vector.tensor_tensor(sl, ub[0:HD, :], rb[0:HD, :],
                                            op=OP.mult)
                    nc.vector.tensor_scalar_add(
                        sl, sl, bias_sb[base:base + HD, bvoff + doh:bvoff + doh + 1])

                if not ATTN_PAIR:
                    et = scores_exp(0)
                    pending = None
                    for h in range(H):
                        if h + 1 < H:
                            net = scores_exp(h + 1)
                        cur = et
                        ud = emit_ud(h, lambda kto: cur[:, kto, :])
                        rdb, ub = emit_recip_ub(ud)
                        if fillers:
                            fillers.pop(0)()
                        if pending is not None:
                            emit_norm(*pending)
                        pending = (h, ub, rdb)
                        if h + 1 < H:
                            et = net
                    emit_norm(*pending)
                else:
                    et = pair_scores_exp(0)
                    for pr in range(H // 2):
                        cur = et
                        items = []
                        for e in range(2):
                            ud = emit_ud(2 * pr + e,
                                         lambda kto, e=e: cur[:, kto, e, :])
                            rdb, ub = emit_recip_ub(ud)
                            items.append((2 * pr + e, ub, rdb))
                        if fillers:
                            fillers.pop(0)()
                        if fillers:
                            fillers.pop(0)()
                        for it in items:
                            emit_norm(*it)
                        if pr + 1 < H // 2:
                            et = pair_scores_exp(pr + 1)
                for f in fillers:
                    f()

            def proj_O(wname, bname, rhs_att):
                """x += W.T @ att + b (residual update)."""
                wt = load_w(wname)
                boff = boffs[bname]
                for do in range(DO):
                    po = psum()
                    for ko in range(DO):
                        nc.tensor.matmul(po[:], lhsT=wt[:, ko, do * P:(do + 1) * P],
                                         rhs=rhs_att[:, ko, :],
                                         start=(ko == 0), stop=(ko == DO - 1))
                    nc.vector.scalar_tensor_tensor(
                        x[:, do, :], po[:], bias_sb[:, boff + do:boff + do + 1],
                        x[:, do, :], op0=OP.add, op1=OP.add)

            # ---------------- full forward pass ----------------
            def body():
                # embedding: gather rows, transpose via PE, scale + pos-enc
                for ti in range(TO):
                    x0 = scp.tile([P, D], F32, tag="x0")
                    nc.gpsimd.indirect_dma_start(
                        out=x0[:], out_offset=None, in_=table[:],
                        in_offset=bass.IndirectOffsetOnAxis(
                            ap=idx_sb[:, ti:ti + 1], axis=0))
                    for do in range(DO):
                        pst = psum()
                        nc.tensor.transpose(pst[:, 0:P], x0[:, do * P:(do + 1) * P],
                                            ident[:])
                        pe_part = scp.tile([P, P], F32, tag="pe")
                        nc.sync.dma_start(pe_part[:],
                                          pe_dram[:, do, ti * P:(ti + 1) * P])
                        nc.vector.scalar_tensor_tensor(
                            x[:, do, ti * P:(ti + 1) * P], pst[:, 0:P],
                            math.sqrt(D), pe_part[:], op0=OP.mult, op1=OP.add)

                hb = act.tile([P, DO, T], B16, tag="hb")
                qt = act.tile([P, DO, T], B16, tag="qt")
                kt = act.tile([P, DO, T], B16, tag="kt", bufs=2)
                att = act.tile([P, DO, T], B16, tag="att")

                for l in range(L):
                    uT = act.tile([P, FO, T], B16, tag="uT", name="uT")
                    # ---- self attention ----
                    layer_norm(3 * l + 0, hb)
                    proj_T(f"saq{l}", f"sabq{l}", hb, qt)
                    proj_T(f"sak{l}", f"sabk{l}", hb, kt)
                    v65 = act.tile([P, TO, H, HD + 1], B16, tag="v65", bufs=2)
                    nc.vector.memset(v65[:, :, :, HD:HD + 1], 1.0)
                    proj_V(f"sav{l}", hb, v65)

                    # cross-attn K/V only depend on the encoder: emit them as
                    # fillers between self-attention heads to keep PE busy.
                    kte = act.tile([P, DO, T], B16, tag="kt", bufs=2)
                    v65e = act.tile([P, TO, H, HD + 1], B16, tag="v65", bufs=2)
                    wke = load_w(f"cak{l}")
                    wve = load_w(f"cav{l}")
                    kboff = boffs[f"cabk{l}"]
                    fillers = []

                    def mk_kenc(do, wke=wke, kte=kte, kboff=kboff):
                        def fill():
                            pq = psum()
                            for ko in range(DO):
                                nc.tensor.matmul(
                                    pq[:], lhsT=wke[:, ko, do * P:(do + 1) * P],
                                    rhs=enc_sb[:, ko, :],
                                    start=(ko == 0), stop=(ko == DO - 1))
                            nc.scalar.activation(
                                kte[:, do, :], pq[:], AF.Identity,
                                bias=bias_sb[:, kboff + do:kboff + do + 1])
                        return fill

                    def mk_venc(to, nh, wve=wve, v65e=v65e):
                        def fill():
                            if to == 0 and nh == 0:
                                nc.vector.memset(v65e[:, :, :, HD:HD + 1], 1.0)
                            pv = psum()
                            for ko in range(DO):
                                nc.tensor.matmul(
                                    pv[:], lhsT=enc_sb[:, ko, to * P:(to + 1) * P],
                                    rhs=wve[:, ko, nh * 512:(nh + 1) * 512],
                                    start=(ko == 0), stop=(ko == DO - 1))
                            nc.vector.tensor_copy(
                                v65e[:, to, nh * 8:(nh + 1) * 8, 0:64],
                                pv.rearrange("p (h d) -> p h d", d=HD))
                        return fill

                    for do in range(DO):
                        fillers.append(mk_kenc(do))
                    for to in range(TO):
                        fillers.append(mk_venc(to, 0))
                    for to in range(TO):
                        fillers.append(mk_venc(to, 1))
                    ln2_fillers = fillers[16 - LN2_FILL:]
                    fillers = fillers[:16 - LN2_FILL]

                    if FILLERS:
                        attention(qt, kt, v65, att, True, f"sabv{l}", fillers)
                        proj_O(f"sao{l}", f"sabo{l}", att)
                        layer_norm(3 * l + 1, hb, fillers=ln2_fillers)
                    else:
                        attention(qt, kt, v65, att, True, f"sabv{l}")
                        proj_O(f"sao{l}", f"sabo{l}", att)
                        for fl in fillers + ln2_fillers:
                            fl()
                        layer_norm(3 * l + 1, hb)
                    proj_T(f"caq{l}", f"cabq{l}", hb, qt)
                    attention(qt, kte, v65e, att, False, f"cabv{l}")
                    proj_O(f"cao{l}", f"cabo{l}", att)

                    # ---- FFN ----
                    layer_norm(3 * l + 2, hb)
                    b1off = boffs[f"b1{l}"]
                    for q in range(4):
                        w1q = load_w1(l, q)
                        for fl in range(8):
                            fo = q * 8 + fl
                            pf = psum()
                            for ko in range(DO):
                                nc.tensor.matmul(
                                    pf[:], lhsT=w1q[:, ko, fl * P:(fl + 1) * P],
                                    rhs=hb[:, ko, :],
                                    start=(ko == 0), stop=(ko == DO - 1))
                            nc.scalar.activation(
                                uT[:, fo, :], pf[:], AF.Relu,
                                bias=bias_sb[:, b1off + fo:b1off + fo + 1])
                    b2off = boffs[f"b2{l}"]
                    for grp in range(2):
                        pys = [psum() for _ in range(4)]
                        for q in range(4):
                            w2q = load_w2(l, q)
                            for dl in range(4):
                                do = grp * 4 + dl
                                for kl in range(8):
                                    fo = q * 8 + kl
                                    nc.tensor.matmul(
                                        pys[dl][:],
                                        lhsT=w2q[:, kl, do * P:(do + 1) * P],
                                        rhs=uT[:, fo, :],
                                        start=(q == 0 and kl == 0),
                                        stop=(q == 3 and kl == 7))
                        for dl in range(4):
                            do = grp * 4 + dl
                            nc.vector.scalar_tensor_tensor(
                                x[:, do, :], pys[dl][:],
                                bias_sb[:, b2off + do:b2off + do + 1],
                                x[:, do, :], op0=OP.add, op1=OP.add)

                # ---- final LN + store ----
                out_sb = act.tile([P, DO, T], F32, tag="uT", name="osb")
                layer_norm(NSLOT - 1, out_sb, final=True)
                nc.sync.dma_start(out_dram[:], out_sb[:])

            for _ in range(repeat):
                body()

    _split_sync_waits(nc)
    return nc


# ------------------------------------------------------------------ entry ---

def kernel(**inputs):
    from concourse.bass_utils import run_bass_kernel_spmd

    nc = build_decoder(repeat=1)
    in_maps = prep_inputs(inputs)
    res = run_bass_kernel_spmd(nc, in_maps, core_ids=list(range(N_CORES)),
                               trace=False)
    return unshard(res.results)



# revision 13
# speedup vs baseline: 1.4761x; 1.4761x over previous
"""Trainium2 Bass kernel for nn_Decoder_64201171141372.

6-layer pre-norm transformer decoder (D=1024, H=16, F=4096, B=8, S=512).
Sharding: data-parallel over batch - each of the 8 NeuronCores computes one
batch element end-to-end; no collectives.

Device-side layout: activations are kept TRANSPOSED in SBUF as [d_part=128,
d_outer, t] so every linear layer runs as matmul(lhsT=W_tile, rhs=xT) with
the contraction dim on partitions.

Host-side algebraic folds (exact):
 - pre-norm LN gamma is folded into the consumer weight rows, LN beta into
   the consumer bias:  W' = diag(g) W,  b' = b + W^T beta.  The device then
   only ever computes the "trivial" LN01(x) = (x - mean) * rstd, whose
   scale/shift rows are broadcast with two rank-1 matmuls shared by all
   d-slices.
 - the K-projection bias is dropped (softmax is invariant to a per-query
   constant), and the V-projection bias is folded into the O bias
   (b_o' = b_o + W_o^T b_v), so attention V rows need no bias either.

LN statistics feed the PE as float32r (no bf16 staging copies); the s1/s2
column-sum matmuls run concurrently in two 32-column PE tiles. Attention
scores for a head pair run concurrently in two 64-row PE tiles. Softmax
denominators use the fast approximate reciprocal (~51 ULP); the normalization
multiply reads numerator and broadcast reciprocal straight from PSUM.
"""

import math

import numpy as np
import ml_dtypes

import concourse.bass as bass
import concourse.mybir as mybir
import concourse.tile as tile
from concourse.masks import make_identity
from concourse.vector_clock import ScopedClock, VectorClock

bf16 = ml_dtypes.bfloat16
F32 = mybir.dt.float32
F32R = mybir.dt.float32r
B16 = mybir.dt.bfloat16
I32 = mybir.dt.int32

P = 128
V, D, H, F, L = 32000, 1024, 16, 4096, 6
T = 512            # decoder seq len == encoder seq len
HD = D // H        # 64
DO = D // P        # 8
FO = F // P        # 32
TO = T // P        # 4
EPS = 1e-5
N_CORES = 8
N_PROCS = 27

AF = mybir.ActivationFunctionType
OP = mybir.AluOpType

# ---- tuning flags ----
X2D_ACT = True         # x^2 tiles on ACT (Square) vs DVE
MASK_POOL = False       # causal mask multiply on gpsimd (Pool) vs DVE
UB_DIRECT = True       # normalization multiply reads ud+rb straight from PSUM
CAUSAL_TRIM = True     # restrict causal blocks to un-masked q range
SA_FILL = 8            # fillers woven into self-attention (one per head pair)
CA_FILL = 4            # next layer's fillers woven into cross-attention
ET_BUFS = 2            # exp-pair tile pipeline depth


class _TC(tile.TileContext):
    """TileContext whose exit drain splits sem waits one per instruction.

    The walrus build in this container only encodes a single sync-wait on
    CTRL-class (Drain/NoOp) instructions; the stock tile exit aggregates one
    wait per logical proc onto one Drain and dies in codegen with "Too many
    sync wait commands". Emit one single-wait NOP per proc on the SP queue
    instead; SP program order then makes the final drain safe with no waits.
    """

    def _drain_and_barrier(self, tick_clock, wait_clock):
        gc = tick_clock.global_clock
        for p in range(N_PROCS):
            t = gc[p]
            if t:
                nop = self.nc.sync.nop(nofuse=True)
                pc = VectorClock([t if q == p else 0 for q in range(N_PROCS)])
                wait_clock.add_sem_waits(nop.ins, ScopedClock({None: pc}))
        self.nc.sync.drain()
        self.nc.all_engine_barrier()
        assert self.sems is not None
        popped = self.nc._tile_sem_poison_stack.pop()
        assert popped is self._sem_poison
        self.nc.clear_and_free_semaphores(list(self.sems.allocated().values()))
        self.nc.all_engine_barrier()


def _split_sync_waits(nc, max_waits=1):
    """Walrus in this container encodes at most one sync-wait per instruction.

    Tile's wait assigner attaches one wait per depended-on logical proc; move
    the extras onto single-wait NoOps inserted just before the instruction on
    the same engine queue (program order preserves the sync semantics).
    """
    n_added = 0
    for f in nc.m.functions:
        for bb in f.blocks:
            insts = bb.instructions
            new_list = []
            for ins in insts:
                si = getattr(ins, "sync_info", None)
                waits = list(si.on_wait) if si is not None and si.on_wait else []
                if len(waits) > max_waits:
                    for w in waits[:-max_waits]:
                        nop = mybir.InstNoOp(
                            name=f"I-wsplit{n_added}", ins=[], outs=[])
                        nop.engine = ins.engine
                        nop.sync_info = mybir.SyncInfo(on_wait=[w], on_update=[])
                        new_list.append(nop)
                        n_added += 1
                    ins.sync_info = mybir.SyncInfo(
                        on_wait=waits[-max_waits:], on_update=list(si.on_update))
                new_list.append(ins)
            if n_added:
                insts[:] = new_list
    return n_added


# ---------------------------------------------------------------- packing ---

def _wpack_offsets():
    """Column offsets (in bf16 elements per partition row) into wpack.

    Each weight W[din, dout] is stored as [128, din//128, dout] flattened on
    the free axis; value at [p, ko, n] = W[ko*128 + p, n].
    """
    offs = {}
    c = 0

    def add(name, ko, n):
        nonlocal c
        offs[name] = (c, ko, n)
        c += ko * n

    for l in range(L):
        for nm, ko, n in [
            ("saq", 8, 1024), ("sak", 8, 1024), ("sav", 8, 1024), ("sao", 8, 1024),
            ("caq", 8, 1024), ("cak", 8, 1024), ("cav", 8, 1024), ("cao", 8, 1024),
            ("w1", 8, 4096), ("w2", 32, 1024),
        ]:
            add(f"{nm}{l}", ko, n)
    return offs, c


def _bpack_offsets():
    """Column offsets into bpack [128, cols] f32: bias b[d] at [d%128, off + d//128]."""
    offs = {}
    c = 0

    def add(name, w):
        nonlocal c
        offs[name] = c
        c += w

    for l in range(L):
        for nm, w in [
            ("sabq", 8), ("sabo", 8), ("cabq", 8), ("cabo", 8),
            ("b1", 32), ("b2", 8),
        ]:
            add(f"{nm}{l}", w)
    return offs, c


def _col_major(w):
    """[din, n] -> [128, din//128, n] with [p, ko, n] = w[ko*128+p, n]."""
    din, n = w.shape
    return np.ascontiguousarray(w.reshape(din // P, P, n).transpose(1, 0, 2))


def _part_cols(b):
    """[d] -> [128, d//128] with [p, o] = b[o*128+p]."""
    return np.ascontiguousarray(b.reshape(-1, P).T)


def prep_inputs(inputs):
    """Host-side packing: returns (in_maps list for 8 cores)."""
    enc = np.asarray(inputs["encoder_output"], np.float32)       # [8, 512, 1024]
    dec = np.asarray(inputs["decoder_input"]).astype(np.int32)   # [8, 512]
    table = np.ascontiguousarray(np.asarray(inputs["embed_table"], np.float32))
    sa_w = np.asarray(inputs["sa_w"], np.float32)
    sa_b = np.asarray(inputs["sa_b"], np.float32)
    ca_w = np.asarray(inputs["ca_w"], np.float32)
    ca_b = np.asarray(inputs["ca_b"], np.float32)
    w1 = np.asarray(inputs["ffn_w1"], np.float32)
    b1 = np.asarray(inputs["ffn_b1"], np.float32)
    w2 = np.asarray(inputs["ffn_w2"], np.float32)
    b2 = np.asarray(inputs["ffn_b2"], np.float32)
    ln_g = np.asarray(inputs["ln_g"], np.float32)
    ln_b = np.asarray(inputs["ln_b"], np.float32)
    fin_g = np.asarray(inputs["final_g"], np.float32)
    fin_b = np.asarray(inputs["final_b"], np.float32)

    woffs, wcols = _wpack_offsets()
    boffs, bcols = _bpack_offsets()
    wpack = np.empty((P, wcols), dtype=bf16)
    bpack = np.zeros((P, bcols), dtype=np.float32)

    def putw(name, w):
        off, ko, n = woffs[name]
        wpack[:, off:off + ko * n] = _col_major(w).reshape(P, -1).astype(bf16)

    def putb(name, b):
        bpack[:, boffs[name]:boffs[name] + len(b) // P] = _part_cols(b)

    for l in range(L):
        g1, be1 = ln_g[l, 0], ln_b[l, 0]
        g2, be2 = ln_g[l, 1], ln_b[l, 1]
        g3, be3 = ln_g[l, 2], ln_b[l, 2]
        # self-attention: fold LN1 gamma/beta; drop K bias; fold V bias into O
        putw(f"saq{l}", sa_w[l, 0] * g1[:, None])
        putb(f"sabq{l}", sa_b[l, 0] + sa_w[l, 0].T @ be1)
        putw(f"sak{l}", sa_w[l, 1] * g1[:, None])
        putw(f"sav{l}", sa_w[l, 2] * g1[:, None])
        bv_eff = sa_b[l, 2] + sa_w[l, 2].T @ be1
        putw(f"sao{l}", sa_w[l, 3])
        putb(f"sabo{l}", sa_b[l, 3] + sa_w[l, 3].T @ bv_eff)
        # cross-attention: K/V consume raw encoder output (no LN fold there)
        putw(f"caq{l}", ca_w[l, 0] * g2[:, None])
        putb(f"cabq{l}", ca_b[l, 0] + ca_w[l, 0].T @ be2)
        putw(f"cak{l}", ca_w[l, 1])
        putw(f"cav{l}", ca_w[l, 2])
        putw(f"cao{l}", ca_w[l, 3])
        putb(f"cabo{l}", ca_b[l, 3] + ca_w[l, 3].T @ ca_b[l, 2])
        # FFN: fold LN3
        putw(f"w1{l}", w1[l] * g3[:, None])
        putb(f"b1{l}", b1[l] + w1[l].T @ be3)
        putw(f"w2{l}", w2[l])
        putb(f"b2{l}", b2[l])

    gfin = np.empty((2, D), dtype=bf16)
    gfin[0] = fin_g
    gfin[1] = fin_b

    # positional encoding, transposed layout [128, 8, 512] fp32
    pos = np.arange(T, dtype=np.float32)[:, None]
    div = np.exp(np.arange(0, D, 2, dtype=np.float32) * (-math.log(10000.0) / D))
    pe = np.zeros((T, D), dtype=np.float32)
    pe[:, 0::2] = np.sin(pos * div)
    pe[:, 1::2] = np.cos(pos * div)
    peT = np.ascontiguousarray(pe.T.reshape(DO, P, T).transpose(1, 0, 2))

    # causal keep-mask for one trimmed wedge: [p, r] = 1.0 if key p <= query r
    cmask = np.ascontiguousarray(
        (np.arange(P)[:, None] <= np.arange(P)[None, :]).astype(bf16))

    in_maps = []
    for c in range(N_CORES):
        encT = np.ascontiguousarray(
            enc[c].T.reshape(DO, P, T).transpose(1, 0, 2)).astype(bf16)
        in_maps.append({
            "wpack": wpack,
            "bpack": bpack,
            "gfin": gfin,
            "table": table,
            "idx": dec[c].copy(),
            "encT": encT,
            "peT": peT,
            "cmask": cmask,
        })
    return in_maps


def unshard(results):
    """Per-core outT [128, 8, 512] -> full [8, 512, 1024] fp32."""
    out = np.empty((N_CORES, T, D), dtype=np.float32)
    for c in range(N_CORES):
        arr = results[c]["out"]                       # [dp, do, t]
        out[c] = arr.transpose(2, 1, 0).reshape(T, D)  # [t, do*128+dp]
    return out


# ----------------------------------------------------------------- device ---

def build_decoder(repeat: int = 1):
    nc = bass.Bass(trn_type="TRN2")
    woffs, wcols = _wpack_offsets()
    boffs, bcols = _bpack_offsets()

    w_dram = nc.dram_tensor("wpack", [P, wcols], B16, kind="ExternalInput")
    b_dram = nc.dram_tensor("bpack", [P, bcols], F32, kind="ExternalInput")
    gf_dram = nc.dram_tensor("gfin", [2, D], B16, kind="ExternalInput")
    table = nc.dram_tensor("table", [V, D], F32, kind="ExternalInput")
    idx_dram = nc.dram_tensor("idx", [T], I32, kind="ExternalInput")
    enc_dram = nc.dram_tensor("encT", [P, DO, T], B16, kind="ExternalInput")
    pe_dram = nc.dram_tensor("peT", [P, DO, T], F32, kind="ExternalInput")
    cm_dram = nc.dram_tensor("cmask", [P, P], B16, kind="ExternalInput")
    out_dram = nc.dram_tensor("out", [P, DO, T], F32, kind="ExternalOutput")

    with _TC(nc) as tc:
        with tc.tile_pool(name="pers", bufs=1) as pers, \
             tc.tile_pool(name="wp", bufs=2) as wp, \
             tc.tile_pool(name="act", bufs=1) as act, \
             tc.tile_pool(name="sc", bufs=2) as scp, \
             tc.tile_pool(name="sm", bufs=1) as sm, \
             tc.tile_pool(name="ps", bufs=8, space="PSUM") as psp:

            # ---- persistent state ----
            x = pers.tile([P, DO, T], F32)       # residual stream (transposed)
            ones = pers.tile([P, 1], B16)
            onesf = pers.tile([P, P], F32)
            ident = pers.tile([P, P], F32)
            bias_sb = pers.tile([P, bcols], F32)
            enc_sb = pers.tile([P, DO, T], B16)
            cm_sb = pers.tile([P, P], B16)
            idx_sb = pers.tile([P, TO], I32)
            gfin_sb = pers.tile([2, D], B16)

            zcol = pers.tile([P, 1], F32)    # zero bias column for ACT ops
            epsD = pers.tile([1, 1], F32)    # D*eps bias for the LN ln()
            hlnD = pers.tile([1, 1], F32)    # +0.5*ln(D) bias for the LN exp()
            sB2 = pers.tile([2, T], F32)     # final-LN shift row + ones row

            nc.vector.memset(ones[:], 1.0)
            nc.vector.memset(onesf[:], 1.0)
            nc.vector.memset(sB2[:], 1.0)
            nc.vector.memset(zcol[:], 0.0)
            nc.vector.memset(epsD[:], float(D) * EPS)
            nc.vector.memset(hlnD[:], 0.5 * math.log(D))
            make_identity(nc, ident[:])
            nc.sync.dma_start(bias_sb[:], b_dram[:])
            nc.sync.dma_start(enc_sb[:], enc_dram[:])
            nc.sync.dma_start(cm_sb[:], cm_dram[:])
            nc.sync.dma_start(gfin_sb[:], gf_dram[:])
            nc.sync.dma_start(idx_sb[:], idx_dram.rearrange("(ti p) -> p ti", p=P))

            def psum(tag="ps"):
                return psp.tile([P, T], F32, tag=tag, name="pt")

            def load_w(name):
                off, ko, n = woffs[name]
                wt = wp.tile([P, 8, 1024], B16, tag="w")
                src = w_dram[:, off:off + ko * n].rearrange("p (o n) -> p o n", o=ko)
                nc.sync.dma_start(wt[:], src)
                return wt

            def load_w1(l, q):
                off, ko, n = woffs[f"w1{l}"]
                wt = wp.tile([P, 8, 1024], B16, tag="w")
                src = w_dram[:, off:off + ko * n].rearrange("p (o n) -> p o n", o=ko)
                nc.sync.dma_start(wt[:], src[:, :, q * 1024:(q + 1) * 1024])
                return wt

            def load_w2(l, q):
                off, ko, n = woffs[f"w2{l}"]
                wt = wp.tile([P, 8, 1024], B16, tag="w")
                src = w_dram[:, off + q * 8192: off + (q + 1) * 8192]
                nc.sync.dma_start(wt[:], src.rearrange("p (o n) -> p o n", o=8))
                return wt

            # ---------------- layer building blocks ----------------
            def layer_norm(out_t, final=False, fillers=()):
                """out_t[:, do, :] = (x - mean) * rstd  (gamma/beta folded into
                consumers; the final LN applies gfin generically)."""
                s1 = psum()
                s2 = psum()
                for do in range(DO):
                    x2d = scp.tile([P, T], B16, tag="x2d", bufs=2, name="x2d")
                    if X2D_ACT:
                        nc.scalar.square(x2d[:], x[:, do, :])
                    else:
                        nc.vector.tensor_tensor(
                            x2d[:], x[:, do, :], x[:, do, :], op=OP.mult)
                    nc.tensor.matmul(s1[0:1, :],
                                     lhsT=onesf[:, 0:1].bitcast(F32R),
                                     rhs=x[:, do, :].bitcast(F32R),
                                     start=(do == 0), stop=(do == DO - 1))
                    nc.tensor.matmul(s2[0:1, :], lhsT=ones[:, 0:1],
                                     rhs=x2d[:],
                                     start=(do == 0), stop=(do == DO - 1))
                for f in fillers:
                    f()
                msq = sm.tile([1, T], F32, tag="lnr", bufs=2, name="msq")
                lnv = sm.tile([1, T], F32, tag="lnr", bufs=2, name="lnv")
                S = sm.tile([1, T], F32, tag="S", bufs=2, name="S")
                # m^2; u = s2 - D*m^2; ln(u + D*eps) = ln(var+eps) + lnD;
                # rstd = exp(-0.5*ln(..) + 0.5*lnD)
                nc.scalar.activation(msq[:], s1[0:1, :], AF.Square,
                                     bias=zcol[0:1, :], scale=1.0 / D)
                nc.vector.scalar_tensor_tensor(msq[:], msq[:], -float(D),
                                               s2[0:1, :],
                                               op0=OP.mult, op1=OP.add)
                nc.scalar.activation(lnv[:], msq[:], AF.Ln, bias=epsD[:])
                nc.scalar.activation(S[:], lnv[:], AF.Exp, scale=-0.5,
                                     bias=hlnD[:])
                nc.vector.scalar_tensor_tensor(sB2[0:1, :], s1[0:1, :],
                                               -1.0 / D, S[:],
                                               op0=OP.mult, op1=OP.mult)
                if not final:
                    pa = psum()
                    pb = psum()
                    nc.tensor.matmul(pa[:], lhsT=onesf[0:1, 0:P].bitcast(F32R),
                                     rhs=S[:].bitcast(F32R),
                                     start=True, stop=True)
                    nc.tensor.matmul(pb[:], lhsT=onesf[0:1, 0:P].bitcast(F32R),
                                     rhs=sB2[0:1, :].bitcast(F32R),
                                     start=True, stop=True)
                    for do in range(DO):
                        nc.vector.tensor_tensor(out_t[:, do, :], x[:, do, :],
                                                pa[:], op=OP.mult)
                        nc.vector.tensor_tensor(out_t[:, do, :], out_t[:, do, :],
                                                pb[:], op=OP.add)
                else:
                    for do in range(DO):
                        A = psum()
                        Bp = psum()
                        nc.tensor.matmul(
                            A[:], lhsT=gfin_sb[0:1, do * P:(do + 1) * P],
                            rhs=S[:].bitcast(F32R), start=True, stop=True)
                        nc.tensor.matmul(
                            Bp[:], lhsT=gfin_sb[0:2, do * P:(do + 1) * P],
                            rhs=sB2[:].bitcast(F32R), start=True, stop=True)
                        nc.vector.tensor_tensor(out_t[:, do, :], x[:, do, :],
                                                A[:], op=OP.mult)
                        nc.vector.tensor_tensor(out_t[:, do, :], out_t[:, do, :],
                                                Bp[:], op=OP.add)

            def proj_T(wname, bname, rhs_t, out_t, ko_outer=False):
                """out_t[dout, t] (transposed layout, bf16) = W.T @ rhs (+ b).

                ko_outer: iterate the contraction dim outermost (groups of 4
                output tiles) so the first matmuls only need rhs slice ko=0 -
                used for the first consumer after a layernorm, whose apply
                produces rhs slices incrementally."""
                wt = load_w(wname)
                boff = boffs[bname] if bname else None

                def epi(out_sl, pq, do):
                    if boff is None:
                        nc.scalar.copy(out_sl, pq[:])
                    else:
                        nc.scalar.activation(
                            out_sl, pq[:], AF.Identity,
                            bias=bias_sb[:, boff + do:boff + do + 1])

                if not ko_outer:
                    for do in range(DO):
                        pq = psum()
                        for ko in range(DO):
                            nc.tensor.matmul(pq[:], lhsT=wt[:, ko, do * P:(do + 1) * P],
                                             rhs=rhs_t[:, ko, :],
                                             start=(ko == 0), stop=(ko == DO - 1))
                        epi(out_t[:, do, :], pq, do)
                else:
                    for grp in range(2):
                        pqs = [psum() for _ in range(4)]
                        for ko in range(DO):
                            for dl in range(4):
                                do = grp * 4 + dl
                                nc.tensor.matmul(
                                    pqs[dl][:], lhsT=wt[:, ko, do * P:(do + 1) * P],
                                    rhs=rhs_t[:, ko, :],
                                    start=(ko == 0), stop=(ko == DO - 1))
                        for dl in range(4):
                            do = grp * 4 + dl
                            epi(out_t[:, do, :], pqs[dl], do)

            def proj_V(wname, rhs_t, v65_t):
                """v65_t[:, to, h, 0:64] = (rhs.T @ Wv) in natural [t, dout] layout."""
                wt = load_w(wname)
                for to in range(TO):
                    for nh in range(2):
                        pv = psum()
                        for ko in range(DO):
                            nc.tensor.matmul(
                                pv[:], lhsT=rhs_t[:, ko, to * P:(to + 1) * P],
                                rhs=wt[:, ko, nh * 512:(nh + 1) * 512],
                                start=(ko == 0), stop=(ko == DO - 1))
                        nc.vector.tensor_copy(
                            v65_t[:, to, nh * 8:(nh + 1) * 8, 0:64],
                            pv.rearrange("p (h d) -> p h d", d=HD))

            def attention(qt_t, kt_t, v65_t, out_att, causal, fillers=()):
                """Pipelined pair-tiled softmax attention (no V/O bias here)."""
                fillers = list(fillers)
                trim = causal and CAUSAL_TRIM

                def q0_of(kto):
                    return kto * P if trim else 0

                def pair_scores_exp(j):
                    """scores + exp (+mask) for head pair (2j, 2j+1)."""
                    et = scp.tile([P, TO, 2, T], B16, tag="expT", bufs=ET_BUFS,
                                  name="et")
                    for kto in range(TO):
                        q0 = q0_of(kto)
                        scs = []
                        for e in range(2):
                            sc = psum()
                            nc.tensor.matmul(
                                sc[:, q0:],
                                lhsT=kt_t[e * HD:(e + 1) * HD, j,
                                          kto * P:(kto + 1) * P],
                                rhs=qt_t[e * HD:(e + 1) * HD, j, q0:],
                                start=True, stop=True)
                            scs.append(sc)
                        for e in range(2):
                            nc.scalar.activation(et[:, kto, e, q0:],
                                                 scs[e][:, q0:], AF.Exp,
                                                 bias=zcol[:],
                                                 scale=1.0 / math.sqrt(HD))
                            if causal:
                                qe = q0 + P if trim else T
                                eng = nc.gpsimd if MASK_POOL else nc.vector
                                eng.tensor_tensor(
                                    et[:, kto, e, q0:qe], et[:, kto, e, q0:qe],
                                    cm_sb[:, 0:qe - q0], op=OP.mult)
                    return et

                def emit_ud(j, e, et):
                    ud = psum()
                    for kto in range(TO):
                        q0 = q0_of(kto)
                        nc.tensor.matmul(ud[0:HD + 1, q0:],
                                         lhsT=v65_t[:, kto, 2 * j + e, :],
                                         rhs=et[:, kto, e, q0:],
                                         start=(kto == 0), stop=(kto == TO - 1))
                    return ud

                def emit_norm(j, e, ud):
                    rdb = sm.tile([1, T], F32, tag="rdb", bufs=2, name="rdb")
                    nc.vector.reciprocal_approx_fast(rdb[:], ud[HD:HD + 1, :])
                    ub = scp.tile([P, T], B16, tag="ub", bufs=2, name="ub")
                    nc.scalar.copy(ub[0:HD, :], ud[0:HD, :])
                    rb = psum()
                    nc.tensor.matmul(rb[0:HD, :],
                                     lhsT=onesf[0:1, 0:HD].bitcast(F32R),
                                     rhs=rdb[:].bitcast(F32R),
                                     start=True, stop=True)
                    sl = out_att[e * HD:(e + 1) * HD, j, :]
                    nc.vector.tensor_tensor(sl, ub[0:HD, :], rb[0:HD, :],
                                            op=OP.mult)

                et = pair_scores_exp(0)
                for j in range(H // 2):
                    cur = et
                    if j + 1 < H // 2:
                        et = pair_scores_exp(j + 1)
                    uds = [emit_ud(j, e, cur) for e in range(2)]
                    if fillers:
                        fillers.pop(0)()
                    for e in range(2):
                        emit_norm(j, e, uds[e])
                for f in fillers:
                    f()

            def proj_O(wname, bname, rhs_att):
                """x += W.T @ att + b (residual update)."""
                wt = load_w(wname)
                boff = boffs[bname]
                for do in range(DO):
                    po = psum()
                    for ko in range(DO):
                        nc.tensor.matmul(po[:], lhsT=wt[:, ko, do * P:(do + 1) * P],
                                         rhs=rhs_att[:, ko, :],
                                         start=(ko == 0), stop=(ko == DO - 1))
                    nc.vector.scalar_tensor_tensor(
                        x[:, do, :], po[:], bias_sb[:, boff + do:boff + do + 1],
                        x[:, do, :], op0=OP.add, op1=OP.add)

            def make_fillers(l):
                """Cross-attn K/V for layer l (encoder-only deps): 16 fillers.

                Each filler DMAs its own weight slice (small dedicated tags) so
                the main `w` rotation never has to survive a layer boundary."""
                kte = act.tile([P, DO, T], B16, tag="kt", bufs=3, name="kte")
                v65e = act.tile([P, TO, H, HD + 1], B16, tag="v65", bufs=2,
                                name="v65e")
                offk, kok, nk = woffs[f"cak{l}"]
                srck = w_dram[:, offk:offk + kok * nk].rearrange(
                    "p (o n) -> p o n", o=kok)
                offv, kov, nv = woffs[f"cav{l}"]
                srcv = w_dram[:, offv:offv + kov * nv].rearrange(
                    "p (o n) -> p o n", o=kov)
                fillers = []

                def mk_kenc(do):
                    def fill():
                        wt = wp.tile([P, DO, P], B16, tag="wfk", bufs=2,
                                     name="wfk")
                        nc.sync.dma_start(wt[:], srck[:, :, do * P:(do + 1) * P])
                        pq = psum()
                        for ko in range(DO):
                            nc.tensor.matmul(
                                pq[:], lhsT=wt[:, ko, :],
                                rhs=enc_sb[:, ko, :],
                                start=(ko == 0), stop=(ko == DO - 1))
                        nc.scalar.copy(kte[:, do, :], pq[:])
                    return fill

                def mk_venc(to, nh):
                    def fill():
                        if to == 0 and nh == 0:
                            nc.vector.memset(v65e[:, :, :, HD:HD + 1], 1.0)
                        wt = wp.tile([P, DO, 512], B16, tag="wfv", bufs=2,
                                     name="wfv")
                        nc.sync.dma_start(
                            wt[:], srcv[:, :, nh * 512:(nh + 1) * 512])
                        pv = psum()
                        for ko in range(DO):
                            nc.tensor.matmul(
                                pv[:], lhsT=enc_sb[:, ko, to * P:(to + 1) * P],
                                rhs=wt[:, ko, :],
                                start=(ko == 0), stop=(ko == DO - 1))
                        nc.vector.tensor_copy(
                            v65e[:, to, nh * 8:(nh + 1) * 8, 0:64],
                            pv.rearrange("p (h d) -> p h d", d=HD))
                    return fill

                for do in range(DO):
                    fillers.append(mk_kenc(do))
                for nh in range(2):
                    for to in range(TO):
                        fillers.append(mk_venc(to, nh))
                return fillers, kte, v65e

            # ---------------- full forward pass ----------------
            def body():
                # embedding: gather rows, transpose via PE, scale + pos-enc
                for ti in range(TO):
                    x0 = scp.tile([P, D], F32, tag="x0", bufs=1)
                    nc.gpsimd.indirect_dma_start(
                        out=x0[:], out_offset=None, in_=table[:],
                        in_offset=bass.IndirectOffsetOnAxis(
                            ap=idx_sb[:, ti:ti + 1], axis=0))
                    for do in range(DO):
                        pst = psum()
                        nc.tensor.transpose(pst[:, 0:P], x0[:, do * P:(do + 1) * P],
                                            ident[:])
                        pe_part = scp.tile([P, P], F32, tag="pe", bufs=1)
                        nc.sync.dma_start(pe_part[:],
                                          pe_dram[:, do, ti * P:(ti + 1) * P])
                        nc.vector.scalar_tensor_tensor(
                            x[:, do, ti * P:(ti + 1) * P], pst[:, 0:P],
                            math.sqrt(D), pe_part[:], op0=OP.mult, op1=OP.add)

                qt = act.tile([P, DO, T], B16, tag="qt")

                fq = []  # pending fillers: (layer, fn); all of layer l must
                         # run before l's cross-attention

                def drain(l):
                    while fq and fq[0][0] <= l:
                        fq.pop(0)[1]()

                def take(n):
                    out = []
                    while fq and len(out) < n:
                        out.append(fq.pop(0)[1])
                    return out

                kv = {}
                for l in range(L):
                    if l == 0:
                        fl, kte, v65e = make_fillers(0)
                        fq.extend((0, f) for f in fl)
                        kv[0] = (kte, v65e)

                    kt = act.tile([P, DO, T], B16, tag="kt", bufs=3, name="kt")
                    uT = act.tile([P, FO, T], B16, tag="uT", name="uT")

                    # ---- self attention ----
                    hb = act.tile([P, DO, T], B16, tag="hb", name="hb1")
                    layer_norm(hb)
                    proj_T(f"saq{l}", f"sabq{l}", hb, qt, ko_outer=True)
                    proj_T(f"sak{l}", None, hb, kt)
                    v65 = act.tile([P, TO, H, HD + 1], B16, tag="v65", bufs=2,
                                   name="v65")
                    nc.vector.memset(v65[:, :, :, HD:HD + 1], 1.0)
                    proj_V(f"sav{l}", hb, v65)

                    att = act.tile([P, DO, T], B16, tag="hb", name="att1")
                    attention(qt, kt, v65, att, True, fillers=take(SA_FILL))
                    proj_O(f"sao{l}", f"sabo{l}", att)
                    hb = act.tile([P, DO, T], B16, tag="hb", name="hb2")
                    layer_norm(hb, fillers=take(99))
                    drain(l)  # safety: all layer-l fillers done pre-CA
                    proj_T(f"caq{l}", f"cabq{l}", hb, qt, ko_outer=True)
                    if l + 1 < L:
                        fl, kte, v65e = make_fillers(l + 1)
                        fq.extend((l + 1, f) for f in fl)
                        kv[l + 1] = (kte, v65e)
                    ktel, v65el = kv.pop(l)
                    att = act.tile([P, DO, T], B16, tag="hb", name="att2")
                    attention(qt, ktel, v65el, att, False, fillers=take(CA_FILL))
                    proj_O(f"cao{l}", f"cabo{l}", att)

                    # ---- FFN ----
                    hb = act.tile([P, DO, T], B16, tag="hb", name="hb3")
                    layer_norm(hb)
                    b1off = boffs[f"b1{l}"]
                    for q in range(4):
                        w1q = load_w1(l, q)
                        for fl_ in range(8):
                            fo = q * 8 + fl_
                            pf = psum()
                            for ko in range(DO):
                                nc.tensor.matmul(
                                    pf[:], lhsT=w1q[:, ko, fl_ * P:(fl_ + 1) * P],
                                    rhs=hb[:, ko, :],
                                    start=(ko == 0), stop=(ko == DO - 1))
                            nc.scalar.activation(
                                uT[:, fo, :], pf[:], AF.Relu,
                                bias=bias_sb[:, b1off + fo:b1off + fo + 1])
                    b2off = boffs[f"b2{l}"]
                    for grp in range(2):
                        pys = [psum() for _ in range(4)]
                        for q in range(4):
                            w2q = load_w2(l, q)
                            for dl in range(4):
                                do = grp * 4 + dl
                                for kl in range(8):
                                    fo = q * 8 + kl
                                    nc.tensor.matmul(
                                        pys[dl][:],
                                        lhsT=w2q[:, kl, do * P:(do + 1) * P],
                                        rhs=uT[:, fo, :],
                                        start=(q == 0 and kl == 0),
                                        stop=(q == 3 and kl == 7))
                        for dl in range(4):
                            do = grp * 4 + dl
                            nc.vector.scalar_tensor_tensor(
                                x[:, do, :], pys[dl][:],
                                bias_sb[:, b2off + do:b2off + do + 1],
                                x[:, do, :], op0=OP.add, op1=OP.add)

                # ---- final LN + store ----
                out_sb = act.tile([P, DO, T], F32, tag="uT", name="osb")
                layer_norm(out_sb, final=True)
                nc.sync.dma_start(out_dram[:], out_sb[:])

            for _ in range(repeat):
                body()

    _split_sync_waits(nc)
    return nc


# ------------------------------------------------------------------ entry ---

def kernel(**inputs):
    from concourse.bass_utils import run_bass_kernel_spmd

    nc = build_decoder(repeat=1)
    in_maps = prep_inputs(inputs)
    res = run_bass_kernel_spmd(nc, in_maps, core_ids=list(range(N_CORES)),
                               trace=False)
    return unshard(res.results)


# revision 15
# speedup vs baseline: 1.5085x; 1.0220x over previous
"""Trainium2 Bass kernel for nn_Decoder_64201171141372.

6-layer pre-norm transformer decoder (D=1024, H=16, F=4096, B=8, S=512).
Sharding: data-parallel over batch - each of the 8 NeuronCores computes one
batch element end-to-end; no collectives.

Device-side layout: activations are kept TRANSPOSED in SBUF as [d_part=128,
d_outer, t] so every linear layer runs as matmul(lhsT=W_tile, rhs=xT) with
the contraction dim on partitions.

Host-side algebraic folds (exact):
 - pre-norm LN gamma is folded into the consumer weight rows, LN beta into
   the consumer bias:  W' = diag(g) W,  b' = b + W^T beta.  The device then
   only ever computes the "trivial" LN01(x) = (x - mean) * rstd, whose
   scale/shift rows are broadcast with two rank-1 matmuls shared by all
   d-slices.
 - the K-projection bias is dropped (softmax is invariant to a per-query
   constant), and the V-projection bias is folded into the O bias
   (b_o' = b_o + W_o^T b_v), so attention V rows need no bias either.

LN statistics feed the PE as float32r (no bf16 staging copies); the s1/s2
column-sum matmuls run concurrently in two 32-column PE tiles. Attention
scores for a head pair run concurrently in two 64-row PE tiles. Softmax
denominators use the fast approximate reciprocal (~51 ULP); the normalization
multiply reads numerator and broadcast reciprocal straight from PSUM.
"""

import math

import numpy as np
import ml_dtypes

import concourse.bass as bass
import concourse.mybir as mybir
import concourse.tile as tile
from concourse.masks import make_identity
from concourse.vector_clock import ScopedClock, VectorClock

bf16 = ml_dtypes.bfloat16
F32 = mybir.dt.float32
F32R = mybir.dt.float32r
B16 = mybir.dt.bfloat16
I32 = mybir.dt.int32

P = 128
V, D, H, F, L = 32000, 1024, 16, 4096, 6
T = 512            # decoder seq len == encoder seq len
HD = D // H        # 64
DO = D // P        # 8
FO = F // P        # 32
TO = T // P        # 4
EPS = 1e-5
N_CORES = 8
N_PROCS = 27

AF = mybir.ActivationFunctionType
OP = mybir.AluOpType

# ---- tuning flags ----
X2D_ACT = True         # x^2 tiles on ACT (Square) vs DVE
MASK_POOL = False       # causal mask multiply on gpsimd (Pool) vs DVE
UB_DIRECT = True       # normalization multiply reads ud+rb straight from PSUM
CAUSAL_TRIM = True     # restrict causal blocks to un-masked q range
SA_FILL = 8            # fillers woven into self-attention (one per head pair)
CA_FILL = 4            # next layer's fillers woven into cross-attention
ET_BUFS = 2            # exp-pair tile pipeline depth


class _TC(tile.TileContext):
    """TileContext whose exit drain splits sem waits one per instruction.

    The walrus build in this container only encodes a single sync-wait on
    CTRL-class (Drain/NoOp) instructions; the stock tile exit aggregates one
    wait per logical proc onto one Drain and dies in codegen with "Too many
    sync wait commands". Emit one single-wait NOP per proc on the SP queue
    instead; SP program order then makes the final drain safe with no waits.
    """

    def _drain_and_barrier(self, tick_clock, wait_clock):
        gc = tick_clock.global_clock
        for p in range(N_PROCS):
            t = gc[p]
            if t:
                nop = self.nc.sync.nop(nofuse=True)
                pc = VectorClock([t if q == p else 0 for q in range(N_PROCS)])
                wait_clock.add_sem_waits(nop.ins, ScopedClock({None: pc}))
        self.nc.sync.drain()
        self.nc.all_engine_barrier()
        assert self.sems is not None
        popped = self.nc._tile_sem_poison_stack.pop()
        assert popped is self._sem_poison
        self.nc.clear_and_free_semaphores(list(self.sems.allocated().values()))
        self.nc.all_engine_barrier()


def _split_sync_waits(nc, max_waits=1):
    """Walrus in this container encodes at most one sync-wait per instruction.

    Tile's wait assigner attaches one wait per depended-on logical proc; move
    the extras onto single-wait NoOps inserted just before the instruction on
    the same engine queue (program order preserves the sync semantics).
    """
    n_added = 0
    for f in nc.m.functions:
        for bb in f.blocks:
            insts = bb.instructions
            new_list = []
            for ins in insts:
                si = getattr(ins, "sync_info", None)
                waits = list(si.on_wait) if si is not None and si.on_wait else []
                if len(waits) > max_waits:
                    for w in waits[:-max_waits]:
                        nop = mybir.InstNoOp(
                            name=f"I-wsplit{n_added}", ins=[], outs=[])
                        nop.engine = ins.engine
                        nop.sync_info = mybir.SyncInfo(on_wait=[w], on_update=[])
                        new_list.append(nop)
                        n_added += 1
                    ins.sync_info = mybir.SyncInfo(
                        on_wait=waits[-max_waits:], on_update=list(si.on_update))
                new_list.append(ins)
            if n_added:
                insts[:] = new_list
    return n_added


# ---------------------------------------------------------------- packing ---

def _wpack_offsets():
    """Column offsets (in bf16 elements per partition row) into wpack.

    Each weight W[din, dout] is stored as [128, din//128, dout] flattened on
    the free axis; value at [p, ko, n] = W[ko*128 + p, n].
    """
    offs = {}
    c = 0

    def add(name, ko, n):
        nonlocal c
        offs[name] = (c, ko, n)
        c += ko * n

    for l in range(L):
        for nm, ko, n in [
            ("saq", 8, 1024), ("sak", 8, 1024), ("sav", 8, 1024), ("sao", 8, 1024),
            ("caq", 8, 1024), ("cak", 8, 1024), ("cav", 8, 1024), ("cao", 8, 1024),
            ("w1", 8, 4096), ("w2", 32, 1024),
        ]:
            add(f"{nm}{l}", ko, n)
    return offs, c


def _bpack_offsets():
    """Column offsets into bpack [128, cols] f32: bias b[d] at [d%128, off + d//128]."""
    offs = {}
    c = 0

    def add(name, w):
        nonlocal c
        offs[name] = c
        c += w

    for l in range(L):
        for nm, w in [
            ("sabq", 8), ("sabo", 8), ("cabq", 8), ("cabo", 8),
            ("b1", 32), ("b2", 8),
        ]:
            add(f"{nm}{l}", w)
    return offs, c


def _col_major(w):
    """[din, n] -> [128, din//128, n] with [p, ko, n] = w[ko*128+p, n]."""
    din, n = w.shape
    return np.ascontiguousarray(w.reshape(din // P, P, n).transpose(1, 0, 2))


def _part_cols(b):
    """[d] -> [128, d//128] with [p, o] = b[o*128+p]."""
    return np.ascontiguousarray(b.reshape(-1, P).T)


def prep_inputs(inputs):
    """Host-side packing: returns (in_maps list for 8 cores)."""
    enc = np.asarray(inputs["encoder_output"], np.float32)       # [8, 512, 1024]
    dec = np.asarray(inputs["decoder_input"]).astype(np.int32)   # [8, 512]
    table = np.ascontiguousarray(np.asarray(inputs["embed_table"], np.float32))
    sa_w = np.asarray(inputs["sa_w"], np.float32)
    sa_b = np.asarray(inputs["sa_b"], np.float32)
    ca_w = np.asarray(inputs["ca_w"], np.float32)
    ca_b = np.asarray(inputs["ca_b"], np.float32)
    w1 = np.asarray(inputs["ffn_w1"], np.float32)
    b1 = np.asarray(inputs["ffn_b1"], np.float32)
    w2 = np.asarray(inputs["ffn_w2"], np.float32)
    b2 = np.asarray(inputs["ffn_b2"], np.float32)
    ln_g = np.asarray(inputs["ln_g"], np.float32)
    ln_b = np.asarray(inputs["ln_b"], np.float32)
    fin_g = np.asarray(inputs["final_g"], np.float32)
    fin_b = np.asarray(inputs["final_b"], np.float32)

    woffs, wcols = _wpack_offsets()
    boffs, bcols = _bpack_offsets()
    wpack = np.empty((P, wcols), dtype=bf16)
    bpack = np.zeros((P, bcols), dtype=np.float32)

    def putw(name, w):
        off, ko, n = woffs[name]
        wpack[:, off:off + ko * n] = _col_major(w).reshape(P, -1).astype(bf16)

    def putb(name, b):
        bpack[:, boffs[name]:boffs[name] + len(b) // P] = _part_cols(b)

    for l in range(L):
        g1, be1 = ln_g[l, 0], ln_b[l, 0]
        g2, be2 = ln_g[l, 1], ln_b[l, 1]
        g3, be3 = ln_g[l, 2], ln_b[l, 2]
        # self-attention: fold LN1 gamma/beta; drop K bias; fold V bias into O
        putw(f"saq{l}", sa_w[l, 0] * g1[:, None])
        putb(f"sabq{l}", sa_b[l, 0] + sa_w[l, 0].T @ be1)
        putw(f"sak{l}", sa_w[l, 1] * g1[:, None])
        putw(f"sav{l}", sa_w[l, 2] * g1[:, None])
        bv_eff = sa_b[l, 2] + sa_w[l, 2].T @ be1
        putw(f"sao{l}", sa_w[l, 3])
        putb(f"sabo{l}", sa_b[l, 3] + sa_w[l, 3].T @ bv_eff)
        # cross-attention: K/V consume raw encoder output (no LN fold there)
        putw(f"caq{l}", ca_w[l, 0] * g2[:, None])
        putb(f"cabq{l}", ca_b[l, 0] + ca_w[l, 0].T @ be2)
        putw(f"cak{l}", ca_w[l, 1])
        putw(f"cav{l}", ca_w[l, 2])
        putw(f"cao{l}", ca_w[l, 3])
        putb(f"cabo{l}", ca_b[l, 3] + ca_w[l, 3].T @ ca_b[l, 2])
        # FFN: fold LN3
        putw(f"w1{l}", w1[l] * g3[:, None])
        putb(f"b1{l}", b1[l] + w1[l].T @ be3)
        putw(f"w2{l}", w2[l])
        putb(f"b2{l}", b2[l])

    gfin = np.empty((2, D), dtype=bf16)
    gfin[0] = fin_g
    gfin[1] = fin_b

    # positional encoding, transposed layout [128, 8, 512] fp32
    pos = np.arange(T, dtype=np.float32)[:, None]
    div = np.exp(np.arange(0, D, 2, dtype=np.float32) * (-math.log(10000.0) / D))
    pe = np.zeros((T, D), dtype=np.float32)
    pe[:, 0::2] = np.sin(pos * div)
    pe[:, 1::2] = np.cos(pos * div)
    peT = np.ascontiguousarray(pe.T.reshape(DO, P, T).transpose(1, 0, 2))

    # causal keep-mask for one trimmed wedge: [p, r] = 1.0 if key p <= query r
    cmask = np.ascontiguousarray(
        (np.arange(P)[:, None] <= np.arange(P)[None, :]).astype(bf16))

    in_maps = []
    for c in range(N_CORES):
        encT = np.ascontiguousarray(
            enc[c].T.reshape(DO, P, T).transpose(1, 0, 2)).astype(bf16)
        in_maps.append({
            "wpack": wpack,
            "bpack": bpack,
            "gfin": gfin,
            "table": table,
            "idx": dec[c].copy(),
            "encT": encT,
            "peT": peT,
            "cmask": cmask,
        })
    return in_maps


def unshard(results):
    """Per-core outT [128, 8, 512] -> full [8, 512, 1024] fp32."""
    out = np.empty((N_CORES, T, D), dtype=np.float32)
    for c in range(N_CORES):
        arr = results[c]["out"]                       # [dp, do, t]
        out[c] = arr.transpose(2, 1, 0).reshape(T, D)  # [t, do*128+dp]
    return out


# ----------------------------------------------------------------- device ---

def build_decoder(repeat: int = 1):
    nc = bass.Bass(trn_type="TRN2")
    woffs, wcols = _wpack_offsets()
    boffs, bcols = _bpack_offsets()

    w_dram = nc.dram_tensor("wpack", [P, wcols], B16, kind="ExternalInput")
    b_dram = nc.dram_tensor("bpack", [P, bcols], F32, kind="ExternalInput")
    gf_dram = nc.dram_tensor("gfin", [2, D], B16, kind="ExternalInput")
    table = nc.dram_tensor("table", [V, D], F32, kind="ExternalInput")
    idx_dram = nc.dram_tensor("idx", [T], I32, kind="ExternalInput")
    enc_dram = nc.dram_tensor("encT", [P, DO, T], B16, kind="ExternalInput")
    pe_dram = nc.dram_tensor("peT", [P, DO, T], F32, kind="ExternalInput")
    cm_dram = nc.dram_tensor("cmask", [P, P], B16, kind="ExternalInput")
    out_dram = nc.dram_tensor("out", [P, DO, T], F32, kind="ExternalOutput")

    with _TC(nc) as tc:
        with tc.tile_pool(name="pers", bufs=1) as pers, \
             tc.tile_pool(name="wp", bufs=2) as wp, \
             tc.tile_pool(name="act", bufs=1) as act, \
             tc.tile_pool(name="sc", bufs=2) as scp, \
             tc.tile_pool(name="sm", bufs=1) as sm, \
             tc.tile_pool(name="ps", bufs=8, space="PSUM") as psp:

            # ---- persistent state ----
            x = pers.tile([P, DO, T], F32)       # residual stream (transposed)
            ones = pers.tile([P, 1], B16)
            onesf = pers.tile([P, P], F32)
            ident = pers.tile([P, P], F32)
            bias_sb = pers.tile([P, bcols], F32)
            enc_sb = pers.tile([P, DO, T], B16)
            cm_sb = pers.tile([P, P], B16)
            idx_sb = pers.tile([P, TO], I32)
            gfin_sb = pers.tile([2, D], B16)

            zcol = pers.tile([P, 1], F32)    # zero bias column for ACT ops
            epsD = pers.tile([1, 1], F32)    # D*eps bias for the LN ln()
            hlnD = pers.tile([1, 1], F32)    # +0.5*ln(D) bias for the LN exp()
            sB2 = pers.tile([2, T], F32)     # final-LN shift row + ones row

            nc.vector.memset(ones[:], 1.0)
            nc.vector.memset(onesf[:], 1.0)
            nc.vector.memset(sB2[:], 1.0)
            nc.vector.memset(zcol[:], 0.0)
            nc.vector.memset(epsD[:], float(D) * EPS)
            nc.vector.memset(hlnD[:], 0.5 * math.log(D))
            make_identity(nc, ident[:])
            nc.sync.dma_start(bias_sb[:], b_dram[:])
            nc.sync.dma_start(enc_sb[:], enc_dram[:])
            nc.sync.dma_start(cm_sb[:], cm_dram[:])
            nc.sync.dma_start(gfin_sb[:], gf_dram[:])
            nc.sync.dma_start(idx_sb[:], idx_dram.rearrange("(ti p) -> p ti", p=P))

            def psum(tag="ps"):
                return psp.tile([P, T], F32, tag=tag, name="pt")

            def load_w(name):
                off, ko, n = woffs[name]
                wt = wp.tile([P, 8, 1024], B16, tag="w")
                src = w_dram[:, off:off + ko * n].rearrange("p (o n) -> p o n", o=ko)
                nc.sync.dma_start(wt[:], src)
                return wt

            def load_w1(l, q):
                off, ko, n = woffs[f"w1{l}"]
                wt = wp.tile([P, 8, 1024], B16, tag="w")
                src = w_dram[:, off:off + ko * n].rearrange("p (o n) -> p o n", o=ko)
                nc.sync.dma_start(wt[:], src[:, :, q * 1024:(q + 1) * 1024])
                return wt

            def load_w2(l, q):
                off, ko, n = woffs[f"w2{l}"]
                wt = wp.tile([P, 8, 1024], B16, tag="w")
                src = w_dram[:, off + q * 8192: off + (q + 1) * 8192]
                nc.sync.dma_start(wt[:], src.rearrange("p (o n) -> p o n", o=8))
                return wt

            # ---------------- layer building blocks ----------------
            def layer_norm(out_t, final=False, fillers=()):
                """out_t[:, do, :] = (x - mean) * rstd  (gamma/beta folded into
                consumers; the final LN applies gfin generically)."""
                s1 = psum()
                s2 = psum()
                for do in range(DO):
                    x2d = scp.tile([P, T], B16, tag="x2d", bufs=2, name="x2d")
                    if X2D_ACT:
                        nc.scalar.square(x2d[:], x[:, do, :])
                    else:
                        nc.vector.tensor_tensor(
                            x2d[:], x[:, do, :], x[:, do, :], op=OP.mult)
                    nc.tensor.matmul(s1[0:1, :],
                                     lhsT=onesf[:, 0:1].bitcast(F32R),
                                     rhs=x[:, do, :].bitcast(F32R),
                                     start=(do == 0), stop=(do == DO - 1),
                                     tile_position=(0, 0))
                    nc.tensor.matmul(s2[32:33, :], lhsT=ones[:, 0:1],
                                     rhs=x2d[:],
                                     start=(do == 0), stop=(do == DO - 1),
                                     tile_position=(0, 32))
                for f in fillers:
                    f()
                msq = sm.tile([1, T], F32, tag="lnr", bufs=2, name="msq")
                lnv = sm.tile([1, T], F32, tag="lnr", bufs=2, name="lnv")
                S = sm.tile([1, T], F32, tag="S", bufs=2, name="S")
                # m^2; u = s2 - D*m^2; ln(u + D*eps) = ln(var+eps) + lnD;
                # rstd = exp(-0.5*ln(..) + 0.5*lnD)
                nc.scalar.activation(msq[:], s1[0:1, :], AF.Square,
                                     bias=zcol[0:1, :], scale=1.0 / D)
                nc.vector.scalar_tensor_tensor(msq[:], msq[:], -float(D),
                                               s2[32:33, :],
                                               op0=OP.mult, op1=OP.add)
                nc.scalar.activation(lnv[:], msq[:], AF.Ln, bias=epsD[:])
                nc.scalar.activation(S[:], lnv[:], AF.Exp, scale=-0.5,
                                     bias=hlnD[:])
                nc.vector.scalar_tensor_tensor(sB2[0:1, :], s1[0:1, :],
                                               -1.0 / D, S[:],
                                               op0=OP.mult, op1=OP.mult)
                if not final:
                    pa = psum()
                    pb = psum()
                    nc.tensor.matmul(pa[:], lhsT=onesf[0:1, 0:P].bitcast(F32R),
                                     rhs=S[:].bitcast(F32R),
                                     start=True, stop=True)
                    nc.tensor.matmul(pb[:], lhsT=onesf[0:1, 0:P].bitcast(F32R),
                                     rhs=sB2[0:1, :].bitcast(F32R),
                                     start=True, stop=True)
                    for do in range(DO):
                        nc.vector.tensor_tensor(out_t[:, do, :], x[:, do, :],
                                                pa[:], op=OP.mult)
                        nc.vector.tensor_tensor(out_t[:, do, :], out_t[:, do, :],
                                                pb[:], op=OP.add)
                else:
                    for do in range(DO):
                        A = psum()
                        Bp = psum()
                        nc.tensor.matmul(
                            A[:], lhsT=gfin_sb[0:1, do * P:(do + 1) * P],
                            rhs=S[:].bitcast(F32R), start=True, stop=True)
                        nc.tensor.matmul(
                            Bp[:], lhsT=gfin_sb[0:2, do * P:(do + 1) * P],
                            rhs=sB2[:].bitcast(F32R), start=True, stop=True)
                        nc.vector.tensor_tensor(out_t[:, do, :], x[:, do, :],
                                                A[:], op=OP.mult)
                        nc.vector.tensor_tensor(out_t[:, do, :], out_t[:, do, :],
                                                Bp[:], op=OP.add)

            def proj_T(wname, bname, rhs_t, out_t, ko_outer=False):
                """out_t[dout, t] (transposed layout, bf16) = W.T @ rhs (+ b).

                ko_outer: iterate the contraction dim outermost (groups of 4
                output tiles) so the first matmuls only need rhs slice ko=0 -
                used for the first consumer after a layernorm, whose apply
                produces rhs slices incrementally."""
                wt = load_w(wname)
                boff = boffs[bname] if bname else None

                def epi(out_sl, pq, do):
                    if boff is None:
                        nc.scalar.copy(out_sl, pq[:])
                    else:
                        nc.scalar.activation(
                            out_sl, pq[:], AF.Identity,
                            bias=bias_sb[:, boff + do:boff + do + 1])

                if not ko_outer:
                    for do in range(DO):
                        pq = psum()
                        for ko in range(DO):
                            nc.tensor.matmul(pq[:], lhsT=wt[:, ko, do * P:(do + 1) * P],
                                             rhs=rhs_t[:, ko, :],
                                             start=(ko == 0), stop=(ko == DO - 1))
                        epi(out_t[:, do, :], pq, do)
                else:
                    for grp in range(2):
                        pqs = [psum() for _ in range(4)]
                        for ko in range(DO):
                            for dl in range(4):
                                do = grp * 4 + dl
                                nc.tensor.matmul(
                                    pqs[dl][:], lhsT=wt[:, ko, do * P:(do + 1) * P],
                                    rhs=rhs_t[:, ko, :],
                                    start=(ko == 0), stop=(ko == DO - 1))
                        for dl in range(4):
                            do = grp * 4 + dl
                            epi(out_t[:, do, :], pqs[dl], do)

            def proj_V(wname, rhs_t, v65_t):
                """v65_t[:, to, h, 0:64] = (rhs.T @ Wv) in natural [t, dout] layout."""
                wt = load_w(wname)
                for to in range(TO):
                    for nh in range(2):
                        pv = psum()
                        for ko in range(DO):
                            nc.tensor.matmul(
                                pv[:], lhsT=rhs_t[:, ko, to * P:(to + 1) * P],
                                rhs=wt[:, ko, nh * 512:(nh + 1) * 512],
                                start=(ko == 0), stop=(ko == DO - 1))
                        nc.vector.tensor_copy(
                            v65_t[:, to, nh * 8:(nh + 1) * 8, 0:64],
                            pv.rearrange("p (h d) -> p h d", d=HD))

            def attention(qt_t, kt_t, v65_t, out_att, causal, fillers=()):
                """Pipelined pair-tiled softmax attention (no V/O bias here)."""
                fillers = list(fillers)
                trim = causal and CAUSAL_TRIM

                def q0_of(kto):
                    return kto * P if trim else 0

                def pair_scores_exp(j):
                    """scores + exp (+mask) for head pair (2j, 2j+1)."""
                    et = scp.tile([P, TO, 2, T], B16, tag="expT", bufs=ET_BUFS,
                                  name="et")
                    for kto in range(TO):
                        q0 = q0_of(kto)
                        scs = []
                        for e in range(2):
                            sc = psum()
                            nc.tensor.matmul(
                                sc[:, q0:],
                                lhsT=kt_t[e * HD:(e + 1) * HD, j,
                                          kto * P:(kto + 1) * P],
                                rhs=qt_t[e * HD:(e + 1) * HD, j, q0:],
                                start=True, stop=True,
                                tile_position=(e * HD, 0))
                            scs.append(sc)
                        for e in range(2):
                            nc.scalar.activation(et[:, kto, e, q0:],
                                                 scs[e][:, q0:], AF.Exp,
                                                 bias=zcol[:],
                                                 scale=1.0 / math.sqrt(HD))
                            if causal:
                                qe = q0 + P if trim else T
                                eng = nc.gpsimd if MASK_POOL else nc.vector
                                eng.tensor_tensor(
                                    et[:, kto, e, q0:qe], et[:, kto, e, q0:qe],
                                    cm_sb[:, 0:qe - q0], op=OP.mult)
                    return et

                def emit_ud(j, e, et):
                    ud = psum()
                    for kto in range(TO):
                        q0 = q0_of(kto)
                        nc.tensor.matmul(ud[0:HD + 1, q0:],
                                         lhsT=v65_t[:, kto, 2 * j + e, :],
                                         rhs=et[:, kto, e, q0:],
                                         start=(kto == 0), stop=(kto == TO - 1))
                    return ud

                def emit_norm(j, e, ud):
                    rdb = sm.tile([1, T], F32, tag="rdb", bufs=2, name="rdb")
                    nc.vector.reciprocal_approx_fast(rdb[:], ud[HD:HD + 1, :])
                    ub = scp.tile([P, T], B16, tag="ub", bufs=2, name="ub")
                    nc.scalar.copy(ub[0:HD, :], ud[0:HD, :])
                    rb = psum()
                    nc.tensor.matmul(rb[0:HD, :],
                                     lhsT=onesf[0:1, 0:HD].bitcast(F32R),
                                     rhs=rdb[:].bitcast(F32R),
                                     start=True, stop=True)
                    sl = out_att[e * HD:(e + 1) * HD, j, :]
                    nc.vector.tensor_tensor(sl, ub[0:HD, :], rb[0:HD, :],
                                            op=OP.mult)

                et = pair_scores_exp(0)
                for j in range(H // 2):
                    cur = et
                    if j + 1 < H // 2:
                        et = pair_scores_exp(j + 1)
                    uds = [emit_ud(j, e, cur) for e in range(2)]
                    if fillers:
                        fillers.pop(0)()
                    for e in range(2):
                        emit_norm(j, e, uds[e])
                for f in fillers:
                    f()

            def proj_O(wname, bname, rhs_att):
                """x += W.T @ att + b (residual update)."""
                wt = load_w(wname)
                boff = boffs[bname]
                for do in range(DO):
                    po = psum()
                    for ko in range(DO):
                        nc.tensor.matmul(po[:], lhsT=wt[:, ko, do * P:(do + 1) * P],
                                         rhs=rhs_att[:, ko, :],
                                         start=(ko == 0), stop=(ko == DO - 1))
                    nc.vector.scalar_tensor_tensor(
                        x[:, do, :], po[:], bias_sb[:, boff + do:boff + do + 1],
                        x[:, do, :], op0=OP.add, op1=OP.add)

            def make_fillers(l):
                """Cross-attn K/V for layer l (encoder-only deps): 16 fillers.

                Each filler DMAs its own weight slice (small dedicated tags) so
                the main `w` rotation never has to survive a layer boundary."""
                kte = act.tile([P, DO, T], B16, tag="kt", bufs=3, name="kte")
                v65e = act.tile([P, TO, H, HD + 1], B16, tag="v65", bufs=2,
                                name="v65e")
                offk, kok, nk = woffs[f"cak{l}"]
                srck = w_dram[:, offk:offk + kok * nk].rearrange(
                    "p (o n) -> p o n", o=kok)
                offv, kov, nv = woffs[f"cav{l}"]
                srcv = w_dram[:, offv:offv + kov * nv].rearrange(
                    "p (o n) -> p o n", o=kov)
                fillers = []

                def mk_kenc(do):
                    def fill():
                        wt = wp.tile([P, DO, P], B16, tag="wfk", bufs=2,
                                     name="wfk")
                        nc.sync.dma_start(wt[:], srck[:, :, do * P:(do + 1) * P])
                        pq = psum()
                        for ko in range(DO):
                            nc.tensor.matmul(
                                pq[:], lhsT=wt[:, ko, :],
                                rhs=enc_sb[:, ko, :],
                                start=(ko == 0), stop=(ko == DO - 1))
                        nc.scalar.copy(kte[:, do, :], pq[:])
                    return fill

                wv_tiles = {}

                def mk_venc(to, nh):
                    def fill():
                        if to == 0 and nh == 0:
                            nc.vector.memset(v65e[:, :, :, HD:HD + 1], 1.0)
                        if nh not in wv_tiles:
                            wt = wp.tile([P, DO, 512], B16, tag="wfv", bufs=2,
                                         name="wfv")
                            nc.sync.dma_start(
                                wt[:], srcv[:, :, nh * 512:(nh + 1) * 512])
                            wv_tiles[nh] = wt
                        wt = wv_tiles[nh]
                        pv = psum()
                        for ko in range(DO):
                            nc.tensor.matmul(
                                pv[:], lhsT=enc_sb[:, ko, to * P:(to + 1) * P],
                                rhs=wt[:, ko, :],
                                start=(ko == 0), stop=(ko == DO - 1))
                        nc.vector.tensor_copy(
                            v65e[:, to, nh * 8:(nh + 1) * 8, 0:64],
                            pv.rearrange("p (h d) -> p h d", d=HD))
                    return fill

                for do in range(DO):
                    fillers.append(mk_kenc(do))
                for nh in range(2):
                    for to in range(TO):
                        fillers.append(mk_venc(to, nh))
                return fillers, kte, v65e

            # ---------------- full forward pass ----------------
            def body():
                # embedding: gather rows, transpose via PE, scale + pos-enc
                for ti in range(TO):
                    x0 = scp.tile([P, D], F32, tag="x0", bufs=1)
                    nc.gpsimd.indirect_dma_start(
                        out=x0[:], out_offset=None, in_=table[:],
                        in_offset=bass.IndirectOffsetOnAxis(
                            ap=idx_sb[:, ti:ti + 1], axis=0))
                    for do in range(DO):
                        pst = psum()
                        nc.tensor.transpose(pst[:, 0:P], x0[:, do * P:(do + 1) * P],
                                            ident[:])
                        pe_part = scp.tile([P, P], F32, tag="pe", bufs=1)
                        nc.sync.dma_start(pe_part[:],
                                          pe_dram[:, do, ti * P:(ti + 1) * P])
                        nc.vector.scalar_tensor_tensor(
                            x[:, do, ti * P:(ti + 1) * P], pst[:, 0:P],
                            math.sqrt(D), pe_part[:], op0=OP.mult, op1=OP.add)

                qt = act.tile([P, DO, T], B16, tag="qt")

                fq = []  # pending fillers: (layer, fn); all of layer l must
                         # run before l's cross-attention

                def drain(l):
                    while fq and fq[0][0] <= l:
                        fq.pop(0)[1]()

                def take(n):
                    out = []
                    while fq and len(out) < n:
                        out.append(fq.pop(0)[1])
                    return out

                kv = {}
                for l in range(L):
                    if l == 0:
                        fl, kte, v65e = make_fillers(0)
                        fq.extend((0, f) for f in fl)
                        kv[0] = (kte, v65e)

                    kt = act.tile([P, DO, T], B16, tag="kt", bufs=3, name="kt")
                    uT = act.tile([P, FO, T], B16, tag="uT", name="uT")

                    # ---- self attention ----
                    hb = act.tile([P, DO, T], B16, tag="hb", name="hb1")
                    layer_norm(hb)
                    proj_T(f"saq{l}", f"sabq{l}", hb, qt, ko_outer=True)
                    proj_T(f"sak{l}", None, hb, kt)
                    v65 = act.tile([P, TO, H, HD + 1], B16, tag="v65", bufs=2,
                                   name="v65")
                    nc.vector.memset(v65[:, :, :, HD:HD + 1], 1.0)
                    proj_V(f"sav{l}", hb, v65)

                    att = act.tile([P, DO, T], B16, tag="hb", name="att1")
                    attention(qt, kt, v65, att, True, fillers=take(SA_FILL))
                    proj_O(f"sao{l}", f"sabo{l}", att)
                    hb = act.tile([P, DO, T], B16, tag="hb", name="hb2")
                    layer_norm(hb, fillers=take(99))
                    drain(l)  # safety: all layer-l fillers done pre-CA
                    proj_T(f"caq{l}", f"cabq{l}", hb, qt, ko_outer=True)
                    if l + 1 < L:
                        fl, kte, v65e = make_fillers(l + 1)
                        fq.extend((l + 1, f) for f in fl)
                        kv[l + 1] = (kte, v65e)
                    ktel, v65el = kv.pop(l)
                    att = act.tile([P, DO, T], B16, tag="hb", name="att2")
                    attention(qt, ktel, v65el, att, False, fillers=take(CA_FILL))
                    proj_O(f"cao{l}", f"cabo{l}", att)

                    # ---- FFN ----
                    hb = act.tile([P, DO, T], B16, tag="hb", name="hb3")
                    layer_norm(hb)
                    b1off = boffs[f"b1{l}"]
                    for q in range(4):
                        w1q = load_w1(l, q)
                        for fl_ in range(8):
                            fo = q * 8 + fl_
                            pf = psum()
                            for ko in range(DO):
                                nc.tensor.matmul(
                                    pf[:], lhsT=w1q[:, ko, fl_ * P:(fl_ + 1) * P],
                                    rhs=hb[:, ko, :],
                                    start=(ko == 0), stop=(ko == DO - 1))
                            nc.scalar.activation(
                                uT[:, fo, :], pf[:], AF.Relu,
                                bias=bias_sb[:, b1off + fo:b1off + fo + 1])
                    b2off = boffs[f"b2{l}"]
                    for grp in range(2):
                        pys = [psum() for _ in range(4)]
                        for q in range(4):
                            w2q = load_w2(l, q)
                            for dl in range(4):
                                do = grp * 4 + dl
                                for kl in range(8):
                                    fo = q * 8 + kl
                                    nc.tensor.matmul(
                                        pys[dl][:],
                                        lhsT=w2q[:, kl, do * P:(do + 1) * P],
                                        rhs=uT[:, fo, :],
                                        start=(q == 0 and kl == 0),
                                        stop=(q == 3 and kl == 7))
                        for dl in range(4):
                            do = grp * 4 + dl
                            nc.vector.scalar_tensor_tensor(
                                x[:, do, :], pys[dl][:],
                                bias_sb[:, b2off + do:b2off + do + 1],
                                x[:, do, :], op0=OP.add, op1=OP.add)

                # ---- final LN + store ----
                out_sb = act.tile([P, DO, T], F32, tag="uT", name="osb")
                layer_norm(out_sb, final=True)
                nc.sync.dma_start(out_dram[:], out_sb[:])

            for _ in range(repeat):
                body()

    _split_sync_waits(nc)
    return nc


# ------------------------------------------------------------------ entry ---

def kernel(**inputs):
    from concourse.bass_utils import run_bass_kernel_spmd

    nc = build_decoder(repeat=1)
    in_maps = prep_inputs(inputs)
    res = run_bass_kernel_spmd(nc, in_maps, core_ids=list(range(N_CORES)),
                               trace=False)
    return unshard(res.results)
